# revision 7
# baseline (speedup 1.0000x reference)
"""ChunkCrossAttention Trainium2 kernel.

Math (per reference):
  x = chunk_embeddings[0]                      # (S, L)
  k, v = split(x @ W_kv.T)                     # (S, D) each
  scores = einsum('jqd,sd->jqs', q, k) / sqrt(D), masked
  attn = softmax(scores, -1)
  out = (attn @ v) @ W_out.T + q  -> LayerNorm(gamma, beta)

Strategy (8 NeuronCores):
  - KV projection sharded over S: each core projects its own 512 keys
    (k^T, v^T in [d, s] layout straight out of the PE).
  - W_out folded into v: v' = v @ W_out.T, with two ones columns appended
    so the attention matmul also emits the softmax denominator.
  - Attention partials: every core computes exp(q_all . k_loc) @ v'_loc
    over its local keys for ALL 8192 query rows (softmax without
    max-subtraction, mask folded into the Exp bias).
  - Partials are exchanged in 4 pipelined WAVES of 2048 query rows each:
    bf16 AllToAll (copy-speed, ~2x a ReduceScatter) + a local f32
    vector reduction + LayerNorm epilogue, all overlapped with the next
    wave's attention compute.  Core c owns global rows
    g*2048 + c*256 + [0,256) of wave g (host reassembles).
  - Attention inner loop is software-pipelined (sc0,sc1,av0,sc2,av1,...)
    so the PE never waits on the scalar engine's Exp.
"""
import sys

sys.path.insert(0, "/opt/trn_rl_repo")

import numpy as np

import concourse.bacc as bacc
import concourse.mybir as mybir
import concourse.tile as tile
from concourse.bass_utils import run_bass_kernel_spmd

N_CORES = 8
J, Q, D = 64, 128, 256
S, L = 4096, 4096
S_LOC = S // N_CORES          # 512 keys per core
QALL = J * Q                  # 8192 query rows total
QR = QALL // N_CORES          # 1024 query rows per core (output shard)
NW = 4                        # waves
NCHUNK = 16                   # q-chunks of 512 rows
WROWS = QALL // NW            # 2048 global rows per wave
RW = WROWS // N_CORES         # 256 rows per core per wave
DP = D + 2                    # attention free dim: D outputs + denom + pad
LN_EPS = 1e-5
SCALE = 1.0 / np.sqrt(D)

F32 = mybir.dt.float32
BF16 = mybir.dt.bfloat16
AF = mybir.ActivationFunctionType
ALU = mybir.AluOpType


def build_program():
    nc = bacc.Bacc(None, num_devices=N_CORES)

    xT = nc.declare_dram_parameter("xT", [L, S_LOC], BF16, isOutput=False)
    wkvT = nc.declare_dram_parameter("wkvT", [L, 2 * D], BF16, isOutput=False)
    qT = nc.declare_dram_parameter("qT", [D, QALL], BF16, isOutput=False)
    qres = nc.declare_dram_parameter("qres", [QR, D], F32, isOutput=False)
    woutT = nc.declare_dram_parameter("woutT", [D, D], BF16, isOutput=False)
    maskb = nc.declare_dram_parameter("maskb", [128, S_LOC // 128], F32,
                                      isOutput=False)
    gamma = nc.declare_dram_parameter("gamma", [D], F32, isOutput=False)
    beta = nc.declare_dram_parameter("beta", [D], F32, isOutput=False)
    y = nc.declare_dram_parameter("y", [QR, D], F32, isOutput=True)

    # per-wave partial exchange buffers:
    # a2a_in[g] slot c = bf16 partials for global q rows g*2048+c*256+[0,256)
    # after AllToAll, a2a_out[g] slot j = core j's partials for OUR rows.
    a2a_in = [nc.dram_tensor(f"a2a_in{g}", [N_CORES, RW, DP], BF16)
              for g in range(NW)]
    a2a_out = [nc.dram_tensor(f"a2a_out{g}", [N_CORES, RW, DP], BF16)
               for g in range(NW)]

    import concourse.bass as bass

    with tile.TileContext(nc) as tc:
        with tc.tile_pool(name="singles", bufs=1) as singles, \
             tc.tile_pool(name="xw", bufs=4) as xw, \
             tc.tile_pool(name="kv", bufs=1) as kvp, \
             tc.tile_pool(name="exp", bufs=4) as epool, \
             tc.tile_pool(name="part", bufs=3) as ppool, \
             tc.tile_pool(name="wave", bufs=2) as wpool, \
             tc.tile_pool(name="small", bufs=8) as small:

            # ---- small constants first (gpsimd queue), then qT waves ----
            woutT_sb = singles.tile([128, 2, D], BF16)
            nc.gpsimd.dma_start(out=woutT_sb,
                                in_=woutT.rearrange("(dc p) d2 -> p dc d2",
                                                    p=128))
            maskb_sb = singles.tile([128, S_LOC // 128], F32)
            nc.gpsimd.dma_start(out=maskb_sb, in_=maskb[:, :])
            g_ap = gamma[:]
            gamma_sb = singles.tile([128, D], F32)
            nc.gpsimd.dma_start(out=gamma_sb, in_=bass.AP(
                tensor=g_ap.tensor, offset=g_ap.offset,
                ap=[[0, 128], g_ap.ap[0]]))
            b_ap = beta[:]
            beta_sb = singles.tile([128, D], F32)
            nc.gpsimd.dma_start(out=beta_sb, in_=bass.AP(
                tensor=b_ap.tensor, offset=b_ap.offset,
                ap=[[0, 128], b_ap.ap[0]]))
            eps_sb = singles.tile([128, 1], F32)
            nc.vector.memset(eps_sb, LN_EPS)

            # qT wave 0 now; waves 1-3 and qres are probe-pinned into the
            # attention phase so they don't steal DMA bandwidth from the
            # x/w stream during the KV projection.
            qT_sb = singles.tile([128, 2, NCHUNK, 512], BF16)
            qres_sb = singles.tile([128, QR // 128, D], F32)

            def load_qT_wave(g):
                nc.gpsimd.dma_start(
                    out=qT_sb[:, :, 4 * g:4 * (g + 1), :],
                    in_=qT[:, g * WROWS:(g + 1) * WROWS].rearrange(
                        "(dc p) q -> p dc q", p=128))

            load_qT_wave(0)

            # ---- phase 1: local K^T / V^T projection over the S shard ----
            # x/w stream on the sync queue; first chunk small so the PE
            # starts as early as possible.
            ps1 = tc.tile_pool(name="ps_kv", bufs=1, space="PSUM")
            ps_kv = ps1.__enter__()
            acc = [ps_kv.tile([128, S_LOC], F32, tag=f"acc{h}", name=f"acc{h}")
                   for h in range(4)]
            chunks = [(0, 128), (128, 384)] + [(512 * i, 512)
                                               for i in range(1, 8)]
            n_mm = sum(nr // 128 for _, nr in chunks) * 4
            mm_i = 0
            for (r0, nr) in chunks:
                na = nr // 128
                xt = xw.tile([128, na, S_LOC], BF16, tag=f"xt{na}")
                nc.sync.dma_start(
                    out=xt,
                    in_=xT[r0:r0 + nr, :].rearrange("(a p) s -> p a s", p=128))
                wt = xw.tile([128, na, 2 * D], BF16, tag=f"wt{na}")
                nc.scalar.dma_start(
                    out=wt,
                    in_=wkvT[r0:r0 + nr, :].rearrange("(a p) s -> p a s",
                                                      p=128))
                for a in range(na):
                    for h in range(4):
                        nc.tensor.matmul(acc[h], wt[:, a, h * 128:(h + 1) * 128],
                                         xt[:, a, :], start=(mm_i == 0),
                                         stop=(mm_i == n_mm - 4 + h))
                    mm_i += 4

            kT_loc = kvp.tile([128, 2, S_LOC], BF16)
            nc.scalar.copy(out=kT_loc[:, 0, :], in_=acc[0])
            nc.scalar.copy(out=kT_loc[:, 1, :], in_=acc[1])
            vT_loc = kvp.tile([128, 2, S_LOC], BF16)
            nc.scalar.copy(out=vT_loc[:, 0, :], in_=acc[2])
            nc.scalar.copy(out=vT_loc[:, 1, :], in_=acc[3])

            # ---- v' = v @ W_out.T, plus ones columns -> [s, DP] ----
            vp_sb = kvp.tile([128, 4, DP], BF16)
            nc.vector.memset(vp_sb, 1.0)
            for ss in range(4):
                pv = ps_kv.tile([128, D], F32, tag="pv", name="pv")
                for dc in range(2):
                    nc.tensor.matmul(
                        pv, vT_loc[:, dc, ss * 128:(ss + 1) * 128],
                        woutT_sb[:, dc, :], start=(dc == 0), stop=(dc == 1))
                nc.vector.tensor_copy(out=vp_sb[:, ss, 0:D], in_=pv)
            ps1.__exit__(None, None, None)

            # ---- phase 2: partial attention, 16 chunks in 4 waves ----
            ps3 = tc.tile_pool(name="ps_at", bufs=1, space="PSUM")
            ps_at = ps3.__enter__()
            ps3b = tc.tile_pool(name="ps_sc", bufs=3, space="PSUM")
            ps_sc = ps3b.__enter__()

            n_st = S_LOC // 128                       # 4 local key tiles

            def scores(p, st):
                sc = ps_sc.tile([128, 512], F32, tag="sc")
                for dc in range(2):
                    nc.tensor.matmul(
                        sc, kT_loc[:, dc, st * 128:(st + 1) * 128],
                        qT_sb[:, dc, p, :], start=(dc == 0), stop=(dc == 1))
                ex = epool.tile([128, 512], BF16, tag="ex")
                nc.scalar.activation(out=ex, in_=sc, func=AF.Exp,
                                     bias=maskb_sb[:, st:st + 1], scale=SCALE)
                return ex

            for p in range(NCHUNK):
                g = p // 4
                at = [ps_at.tile([128, DP], F32, tag=f"at{i}", name=f"at{i}")
                      for i in range(4)]

                # software pipeline: sc0, sc1, av0, sc2, av1, sc3, av2, av3
                ex = [None] * n_st

                def av(st):
                    for qt in range(4):
                        nc.tensor.matmul(
                            at[qt], ex[st][:, qt * 128:(qt + 1) * 128],
                            vp_sb[:, st, :],
                            start=(st == 0), stop=(st == n_st - 1))

                ex[0] = scores(p, 0)
                ex[1] = scores(p, 1)
                av(0)
                ex[2] = scores(p, 2)
                av(1)
                ex[3] = scores(p, 3)
                av(2)
                av(3)

                part = ppool.tile([128, 4, DP], BF16, tag="part")
                for qt in range(4):
                    nc.vector.tensor_copy(out=part[:, qt, :], in_=at[qt])
                a2a_r = a2a_in[g].rearrange("s (t p) f -> s p t f", p=128)
                slot = 2 * (p % 4)
                nc.scalar.dma_start(out=a2a_r[slot], in_=part[:, 0:2, :])
                nc.scalar.dma_start(out=a2a_r[slot + 1], in_=part[:, 2:4, :])

                # probe-pinned late loads: one vector op reads BOTH the part
                # tile just produced by this chunk (RAW dep — pins the probe
                # into the attention pipeline; tile reorders anything weaker)
                # and the DMA's target region (WAR dep — pins the transfer
                # behind the probe).  Keeps the x/w stream alone on HBM
                # during the KV projection.
                if p in (0, 2, 4, 8):
                    if p == 2:
                        qprobe = small.tile([128, 1], F32, tag="qprobe")
                        nc.vector.tensor_add(out=qprobe,
                                             in0=qres_sb[:, 0, 0:1],
                                             in1=part[:, 0, 0:1])
                        nc.gpsimd.dma_start(
                            out=qres_sb,
                            in_=qres.rearrange("(t p) d -> p t d", p=128))
                    else:
                        gload = {0: 1, 4: 2, 8: 3}[p]
                        probe = small.tile([128, 1], BF16, tag="probe")
                        nc.vector.tensor_add(
                            out=probe, in0=qT_sb[:, 0, 4 * gload, 0:1],
                            in1=part[:, 0, 0:1])
                        load_qT_wave(gload)

                if p % 4 == 3:
                    nc.gpsimd.collective_compute(
                        "AllToAll", ALU.bypass,
                        replica_groups=[list(range(N_CORES))],
                        ins=[a2a_in[g][:, :, :]], outs=[a2a_out[g][:, :, :]])

            ps3b.__exit__(None, None, None)
            ps3.__exit__(None, None, None)

            # ---- phase 3: per-wave local reduce + epilogue ----
            y_r = y.rearrange("(w t p) d -> w p t d", w=NW, p=128)
            for g in range(NW):
                ao = wpool.tile([128, N_CORES * 2, DP], BF16, tag="ao")
                nc.scalar.dma_start(
                    out=ao,
                    in_=a2a_out[g].rearrange("s (t p) f -> p (s t) f", p=128))
                red = wpool.tile([128, 2, DP], F32, tag="red")
                # view [p, t, f, s]: slot index s has stride 2*DP in ao
                ao_v = ao.rearrange("p (s t) f -> p t f s", s=N_CORES)
                nc.vector.tensor_reduce(
                    out=red.rearrange("p t f -> p (t f)"), in_=ao_v,
                    axis=mybir.AxisListType.X, op=ALU.add)
                h_half = wpool.tile([128, 2, D], F32, tag="h")
                for t in range(2):
                    hs = h_half[:, t, :]
                    rec = small.tile([128, 1], F32, tag="rec")
                    nc.vector.reciprocal(out=rec, in_=red[:, t, D:D + 1])
                    nc.vector.tensor_scalar_mul(out=hs, in0=red[:, t, 0:D],
                                                scalar1=rec)
                    nc.vector.tensor_add(out=hs, in0=hs,
                                         in1=qres_sb[:, 2 * g + t, :])
                    stats = small.tile([128, 6], F32, tag="stats")
                    nc.vector.bn_stats(out=stats, in_=hs)
                    mv = small.tile([128, 2], F32, tag="mv")
                    nc.vector.bn_aggr(out=mv, in_=stats)
                    rstd = small.tile([128, 1], F32, tag="rstd")
                    nc.scalar.activation(out=rstd, in_=mv[:, 1:2], func=AF.Sqrt,
                                         bias=eps_sb, scale=1.0)
                    nc.vector.reciprocal(out=rstd, in_=rstd)
                    nc.vector.tensor_scalar(out=hs, in0=hs,
                                            scalar1=mv[:, 0:1], scalar2=rstd,
                                            op0=ALU.subtract, op1=ALU.mult)
                    nc.vector.tensor_mul(out=hs, in0=hs, in1=gamma_sb)
                    nc.vector.tensor_add(out=hs, in0=hs, in1=beta_sb)
                nc.scalar.dma_start(out=y_r[g], in_=h_half)

    nc.finalize()
    return nc


_NC_CACHE = None


def _make_in_maps(inputs):
    jq = np.asarray(inputs["justice_queries"], dtype=np.float32)
    x = np.asarray(inputs["chunk_embeddings"], dtype=np.float32)[0]
    mask = np.asarray(inputs["chunk_mask"])
    wkv = np.asarray(inputs["W_kv"], dtype=np.float32)
    wout = np.asarray(inputs["W_out"], dtype=np.float32)
    gamma = np.asarray(inputs["ln_gamma"], dtype=np.float32)
    beta = np.asarray(inputs["ln_beta"], dtype=np.float32)

    import ml_dtypes
    bf16 = ml_dtypes.bfloat16
    xT = np.ascontiguousarray(x.T.astype(bf16))         # (L, S)
    wkvT = np.ascontiguousarray(wkv.T.astype(bf16))     # (L, 2D)
    flat = np.ascontiguousarray(jq.reshape(J * Q, D))   # (8192, D)
    qT = np.ascontiguousarray(flat.T.astype(bf16))      # (D, 8192)
    woutT = np.ascontiguousarray(wout.T.astype(bf16))   # (D, D)
    mb_full = np.where(mask != 0, 0.0, -1e30).astype(np.float32)

    in_maps = []
    for c in range(N_CORES):
        mb = mb_full[c * S_LOC:(c + 1) * S_LOC]
        # core c owns global rows g*2048 + c*256 + [0,256) of wave g
        rows = np.concatenate([
            np.arange(g * WROWS + c * RW, g * WROWS + (c + 1) * RW)
            for g in range(NW)])
        in_maps.append({
            "xT": np.ascontiguousarray(xT[:, c * S_LOC:(c + 1) * S_LOC]),
            "wkvT": wkvT,
            "qT": qT,
            "qres": np.ascontiguousarray(flat[rows, :]),
            "woutT": woutT,
            "maskb": np.ascontiguousarray(mb.reshape(S_LOC // 128, 128).T),
            "gamma": gamma,
            "beta": beta,
        })
    return in_maps


def kernel(**inputs) -> np.ndarray:
    global _NC_CACHE
    in_maps = _make_in_maps(inputs)
    if _NC_CACHE is None:
        _NC_CACHE = build_program()
    res = run_bass_kernel_spmd(_NC_CACHE, in_maps, list(range(N_CORES)))
    out = np.empty((QALL, D), dtype=np.float32)
    for c in range(N_CORES):
        yc = res.results[c]["y"]
        for g in range(NW):
            out[g * WROWS + c * RW:g * WROWS + (c + 1) * RW] = \
                yc[g * RW:(g + 1) * RW]
    return np.ascontiguousarray(out.reshape(J, Q, D))


# revision 11
# speedup vs baseline: 1.0014x; 1.0014x over previous
"""ChunkCrossAttention Trainium2 kernel.

Math (per reference):
  x = chunk_embeddings[0]                      # (S, L)
  k, v = split(x @ W_kv.T)                     # (S, D) each
  scores = einsum('jqd,sd->jqs', q, k) / sqrt(D), masked
  attn = softmax(scores, -1)
  out = (attn @ v) @ W_out.T + q  -> LayerNorm(gamma, beta)

Strategy (8 NeuronCores):
  - KV projection sharded over S: each core projects its own 512 keys
    (k^T, v^T in [d, s] layout straight out of the PE).
  - W_out folded into v: v' = v @ W_out.T, with two ones columns appended
    so the attention matmul also emits the softmax denominator.
  - Attention partials: every core computes exp(q_all . k_loc) @ v'_loc
    over its local keys for ALL 8192 query rows (softmax without
    max-subtraction, mask folded into the Exp bias).
  - Partials are exchanged in 4 pipelined WAVES of 2048 query rows each:
    bf16 AllToAll (copy-speed, ~2x a ReduceScatter) + a local f32
    vector reduction + LayerNorm epilogue, all overlapped with the next
    wave's attention compute.  Core c owns global rows
    g*2048 + c*256 + [0,256) of wave g (host reassembles).
  - Attention inner loop is software-pipelined (sc0,sc1,av0,sc2,av1,...)
    so the PE never waits on the scalar engine's Exp.
"""
import sys

sys.path.insert(0, "/opt/trn_rl_repo")

import numpy as np

import concourse.bacc as bacc
import concourse.mybir as mybir
import concourse.tile as tile
from concourse.bass_utils import run_bass_kernel_spmd

N_CORES = 8
J, Q, D = 64, 128, 256
S, L = 4096, 4096
S_LOC = S // N_CORES          # 512 keys per core
QALL = J * Q                  # 8192 query rows total
QR = QALL // N_CORES          # 1024 query rows per core (output shard)
NW = 4                        # waves
NCHUNK = 16                   # q-chunks of 512 rows
WROWS = QALL // NW            # 2048 global rows per wave
RW = WROWS // N_CORES         # 256 rows per core per wave
DP = D + 2                    # attention free dim: D outputs + denom + pad
LN_EPS = 1e-5
SCALE = 1.0 / np.sqrt(D)

F32 = mybir.dt.float32
BF16 = mybir.dt.bfloat16
AF = mybir.ActivationFunctionType
ALU = mybir.AluOpType


def build_program():
    nc = bacc.Bacc(None, num_devices=N_CORES)

    xT = nc.declare_dram_parameter("xT", [L, S_LOC], BF16, isOutput=False)
    wkvT = nc.declare_dram_parameter("wkvT", [L, 2 * D], BF16, isOutput=False)
    qT = nc.declare_dram_parameter("qT", [D, QALL], BF16, isOutput=False)
    qres = nc.declare_dram_parameter("qres", [QR, D], F32, isOutput=False)
    woutT = nc.declare_dram_parameter("woutT", [D, D], BF16, isOutput=False)
    maskb = nc.declare_dram_parameter("maskb", [128, S_LOC // 128], F32,
                                      isOutput=False)
    gamma = nc.declare_dram_parameter("gamma", [D], F32, isOutput=False)
    beta = nc.declare_dram_parameter("beta", [D], F32, isOutput=False)
    y = nc.declare_dram_parameter("y", [QR, D], F32, isOutput=True)

    # per-wave partial exchange buffers:
    # a2a_in[g] slot c = bf16 partials for global q rows g*2048+c*256+[0,256)
    # after AllToAll, a2a_out[g] slot j = core j's partials for OUR rows.
    a2a_in = [nc.dram_tensor(f"a2a_in{g}", [N_CORES, RW, DP], BF16)
              for g in range(NW)]
    a2a_out = [nc.dram_tensor(f"a2a_out{g}", [N_CORES, RW, DP], BF16)
               for g in range(NW)]
    # tiny warmup collective: absorbs the first-collective channel-setup
    # cost (~20us) during phase 1 instead of on the critical path
    warm_in = nc.dram_tensor("warm_in", [N_CORES, 16], BF16)
    warm_out = nc.dram_tensor("warm_out", [N_CORES, 16], BF16)

    import concourse.bass as bass

    with tile.TileContext(nc) as tc:
        with tc.tile_pool(name="singles", bufs=1) as singles, \
             tc.tile_pool(name="xw", bufs=4) as xw, \
             tc.tile_pool(name="kv", bufs=1) as kvp, \
             tc.tile_pool(name="exp", bufs=4) as epool, \
             tc.tile_pool(name="part", bufs=3) as ppool, \
             tc.tile_pool(name="wave", bufs=2) as wpool, \
             tc.tile_pool(name="small", bufs=8) as small:

            # ---- small constants first (gpsimd queue), then qT waves ----
            woutT_sb = singles.tile([128, 2, D], BF16)
            nc.gpsimd.dma_start(out=woutT_sb,
                                in_=woutT.rearrange("(dc p) d2 -> p dc d2",
                                                    p=128))
            maskb_sb = singles.tile([128, S_LOC // 128], F32)
            nc.gpsimd.dma_start(out=maskb_sb, in_=maskb[:, :])
            g_ap = gamma[:]
            gamma_sb = singles.tile([128, D], F32)
            nc.gpsimd.dma_start(out=gamma_sb, in_=bass.AP(
                tensor=g_ap.tensor, offset=g_ap.offset,
                ap=[[0, 128], g_ap.ap[0]]))
            b_ap = beta[:]
            beta_sb = singles.tile([128, D], F32)
            nc.gpsimd.dma_start(out=beta_sb, in_=bass.AP(
                tensor=b_ap.tensor, offset=b_ap.offset,
                ap=[[0, 128], b_ap.ap[0]]))
            eps_sb = singles.tile([128, 1], F32)
            nc.vector.memset(eps_sb, LN_EPS)

            # qT wave 0 now; waves 1-3 and qres are probe-pinned into the
            # attention phase so they don't steal DMA bandwidth from the
            # x/w stream during the KV projection.
            qT_sb = singles.tile([128, 2, NCHUNK, 512], BF16)
            qres_sb = singles.tile([128, QR // 128, D], F32)

            def load_qT_wave(g):
                nc.gpsimd.dma_start(
                    out=qT_sb[:, :, 4 * g:4 * (g + 1), :],
                    in_=qT[:, g * WROWS:(g + 1) * WROWS].rearrange(
                        "(dc p) q -> p dc q", p=128))

            load_qT_wave(0)

            warm_sb = small.tile([128, 1], BF16, tag="warm")
            nc.vector.memset(warm_sb, 0.0)
            nc.gpsimd.dma_start(out=warm_in.rearrange("s w -> (s w)"),
                                in_=warm_sb)
            nc.gpsimd.collective_compute(
                "AllToAll", ALU.bypass,
                replica_groups=[list(range(N_CORES))],
                ins=[warm_in[:, :]], outs=[warm_out[:, :]])

            # ---- phase 1: local K^T / V^T projection over the S shard ----
            # x/w stream on the sync queue; first chunk small so the PE
            # starts as early as possible.
            ps1 = tc.tile_pool(name="ps_kv", bufs=1, space="PSUM")
            ps_kv = ps1.__enter__()
            acc = [ps_kv.tile([128, S_LOC], F32, tag=f"acc{h}", name=f"acc{h}")
                   for h in range(4)]
            chunks = [(0, 128), (128, 384)] + [(512 * i, 512)
                                               for i in range(1, 8)]
            n_mm = sum(nr // 128 for _, nr in chunks) * 4
            mm_i = 0
            for (r0, nr) in chunks:
                na = nr // 128
                xt = xw.tile([128, na, S_LOC], BF16, tag=f"xt{na}")
                nc.sync.dma_start(
                    out=xt,
                    in_=xT[r0:r0 + nr, :].rearrange("(a p) s -> p a s", p=128))
                wt = xw.tile([128, na, 2 * D], BF16, tag=f"wt{na}")
                nc.scalar.dma_start(
                    out=wt,
                    in_=wkvT[r0:r0 + nr, :].rearrange("(a p) s -> p a s",
                                                      p=128))
                for a in range(na):
                    for h in range(4):
                        nc.tensor.matmul(acc[h], wt[:, a, h * 128:(h + 1) * 128],
                                         xt[:, a, :], start=(mm_i == 0),
                                         stop=(mm_i == n_mm - 4 + h))
                    mm_i += 4

            kT_loc = kvp.tile([128, 2, S_LOC], BF16)
            nc.scalar.copy(out=kT_loc[:, 0, :], in_=acc[0])
            nc.scalar.copy(out=kT_loc[:, 1, :], in_=acc[1])
            vT_loc = kvp.tile([128, 2, S_LOC], BF16)
            nc.scalar.copy(out=vT_loc[:, 0, :], in_=acc[2])
            nc.scalar.copy(out=vT_loc[:, 1, :], in_=acc[3])

            # ---- v' = v @ W_out.T, plus ones columns -> [s, DP] ----
            vp_sb = kvp.tile([128, 4, DP], BF16)
            nc.vector.memset(vp_sb, 1.0)
            for ss in range(4):
                pv = ps_kv.tile([128, D], F32, tag="pv", name="pv")
                for dc in range(2):
                    nc.tensor.matmul(
                        pv, vT_loc[:, dc, ss * 128:(ss + 1) * 128],
                        woutT_sb[:, dc, :], start=(dc == 0), stop=(dc == 1))
                nc.vector.tensor_copy(out=vp_sb[:, ss, 0:D], in_=pv)
            ps1.__exit__(None, None, None)

            # ---- phase 2: partial attention, 16 chunks in 4 waves ----
            ps3 = tc.tile_pool(name="ps_at", bufs=1, space="PSUM")
            ps_at = ps3.__enter__()
            ps3b = tc.tile_pool(name="ps_sc", bufs=3, space="PSUM")
            ps_sc = ps3b.__enter__()

            n_st = S_LOC // 128                       # 4 local key tiles

            def scores(p, st):
                sc = ps_sc.tile([128, 512], F32, tag="sc")
                for dc in range(2):
                    nc.tensor.matmul(
                        sc, kT_loc[:, dc, st * 128:(st + 1) * 128],
                        qT_sb[:, dc, p, :], start=(dc == 0), stop=(dc == 1))
                ex = epool.tile([128, 512], BF16, tag="ex")
                nc.scalar.activation(out=ex, in_=sc, func=AF.Exp,
                                     bias=maskb_sb[:, st:st + 1], scale=SCALE)
                return ex

            for p in range(NCHUNK):
                g = p // 4
                at = [ps_at.tile([128, DP], F32, tag=f"at{i}", name=f"at{i}")
                      for i in range(4)]

                # software pipeline: sc0, sc1, av0, sc2, av1, sc3, av2, av3
                ex = [None] * n_st

                def av(st):
                    for qt in range(4):
                        nc.tensor.matmul(
                            at[qt], ex[st][:, qt * 128:(qt + 1) * 128],
                            vp_sb[:, st, :],
                            start=(st == 0), stop=(st == n_st - 1))

                ex[0] = scores(p, 0)
                ex[1] = scores(p, 1)
                av(0)
                ex[2] = scores(p, 2)
                av(1)
                ex[3] = scores(p, 3)
                av(2)
                av(3)

                part = ppool.tile([128, 4, DP], BF16, tag="part")
                for qt in range(4):
                    nc.vector.tensor_copy(out=part[:, qt, :], in_=at[qt])
                a2a_r = a2a_in[g].rearrange("s (t p) f -> s p t f", p=128)
                slot = 2 * (p % 4)
                nc.scalar.dma_start(out=a2a_r[slot], in_=part[:, 0:2, :])
                nc.scalar.dma_start(out=a2a_r[slot + 1], in_=part[:, 2:4, :])

                # probe-pinned late loads: one vector op reads BOTH the part
                # tile just produced by this chunk (RAW dep — pins the probe
                # into the attention pipeline; tile reorders anything weaker)
                # and the DMA's target region (WAR dep — pins the transfer
                # behind the probe).  Keeps the x/w stream alone on HBM
                # during the KV projection.
                if p in (0, 1, 2):
                    if p == 2:
                        qprobe = small.tile([128, 1], F32, tag="qprobe")
                        nc.vector.tensor_add(out=qprobe,
                                             in0=qres_sb[:, 0, 0:1],
                                             in1=part[:, 0, 0:1])
                        nc.gpsimd.dma_start(
                            out=qres_sb,
                            in_=qres.rearrange("(t p) d -> p t d", p=128))
                    elif p == 0:
                        probe = small.tile([128, 1], BF16, tag="probe")
                        nc.vector.tensor_add(
                            out=probe, in0=qT_sb[:, 0, 4, 0:1],
                            in1=part[:, 0, 0:1])
                        load_qT_wave(1)
                    else:
                        for gload in (2, 3):
                            probe = small.tile([128, 1], BF16, tag="probe")
                            nc.vector.tensor_add(
                                out=probe, in0=qT_sb[:, 0, 4 * gload, 0:1],
                                in1=part[:, 0, 0:1])
                            load_qT_wave(gload)

                if p % 4 == 3:
                    nc.gpsimd.collective_compute(
                        "AllToAll", ALU.bypass,
                        replica_groups=[list(range(N_CORES))],
                        ins=[a2a_in[g][:, :, :]], outs=[a2a_out[g][:, :, :]])

            ps3b.__exit__(None, None, None)
            ps3.__exit__(None, None, None)

            # ---- phase 3: per-wave local reduce + epilogue ----
            y_r = y.rearrange("(w t p) d -> w p t d", w=NW, p=128)
            for g in range(NW):
                ao = wpool.tile([128, N_CORES * 2, DP], BF16, tag="ao")
                nc.scalar.dma_start(
                    out=ao,
                    in_=a2a_out[g].rearrange("s (t p) f -> p (s t) f", p=128))
                red = wpool.tile([128, 2, DP], F32, tag="red")
                # view [p, t, f, s]: slot index s has stride 2*DP in ao
                ao_v = ao.rearrange("p (s t) f -> p t f s", s=N_CORES)
                nc.vector.tensor_reduce(
                    out=red.rearrange("p t f -> p (t f)"), in_=ao_v,
                    axis=mybir.AxisListType.X, op=ALU.add)
                h_half = wpool.tile([128, 2, D], F32, tag="h")
                for t in range(2):
                    hs = h_half[:, t, :]
                    rec = small.tile([128, 1], F32, tag="rec")
                    nc.vector.reciprocal(out=rec, in_=red[:, t, D:D + 1])
                    nc.vector.tensor_scalar_mul(out=hs, in0=red[:, t, 0:D],
                                                scalar1=rec)
                    nc.vector.tensor_add(out=hs, in0=hs,
                                         in1=qres_sb[:, 2 * g + t, :])
                    stats = small.tile([128, 6], F32, tag="stats")
                    nc.vector.bn_stats(out=stats, in_=hs)
                    mv = small.tile([128, 2], F32, tag="mv")
                    nc.vector.bn_aggr(out=mv, in_=stats)
                    rstd = small.tile([128, 1], F32, tag="rstd")
                    nc.scalar.activation(out=rstd, in_=mv[:, 1:2], func=AF.Sqrt,
                                         bias=eps_sb, scale=1.0)
                    nc.vector.reciprocal(out=rstd, in_=rstd)
                    nc.vector.tensor_scalar(out=hs, in0=hs,
                                            scalar1=mv[:, 0:1], scalar2=rstd,
                                            op0=ALU.subtract, op1=ALU.mult)
                    nc.vector.tensor_mul(out=hs, in0=hs, in1=gamma_sb)
                    nc.vector.tensor_add(out=hs, in0=hs, in1=beta_sb)
                nc.scalar.dma_start(out=y_r[g], in_=h_half)

    nc.finalize()
    return nc


_NC_CACHE = None


def _make_in_maps(inputs):
    jq = np.asarray(inputs["justice_queries"], dtype=np.float32)
    x = np.asarray(inputs["chunk_embeddings"], dtype=np.float32)[0]
    mask = np.asarray(inputs["chunk_mask"])
    wkv = np.asarray(inputs["W_kv"], dtype=np.float32)
    wout = np.asarray(inputs["W_out"], dtype=np.float32)
    gamma = np.asarray(inputs["ln_gamma"], dtype=np.float32)
    beta = np.asarray(inputs["ln_beta"], dtype=np.float32)

    import ml_dtypes
    bf16 = ml_dtypes.bfloat16
    xT = np.ascontiguousarray(x.T.astype(bf16))         # (L, S)
    wkvT = np.ascontiguousarray(wkv.T.astype(bf16))     # (L, 2D)
    flat = np.ascontiguousarray(jq.reshape(J * Q, D))   # (8192, D)
    qT = np.ascontiguousarray(flat.T.astype(bf16))      # (D, 8192)
    woutT = np.ascontiguousarray(wout.T.astype(bf16))   # (D, D)
    mb_full = np.where(mask != 0, 0.0, -1e30).astype(np.float32)

    in_maps = []
    for c in range(N_CORES):
        mb = mb_full[c * S_LOC:(c + 1) * S_LOC]
        # core c owns global rows g*2048 + c*256 + [0,256) of wave g
        rows = np.concatenate([
            np.arange(g * WROWS + c * RW, g * WROWS + (c + 1) * RW)
            for g in range(NW)])
        in_maps.append({
            "xT": np.ascontiguousarray(xT[:, c * S_LOC:(c + 1) * S_LOC]),
            "wkvT": wkvT,
            "qT": qT,
            "qres": np.ascontiguousarray(flat[rows, :]),
            "woutT": woutT,
            "maskb": np.ascontiguousarray(mb.reshape(S_LOC // 128, 128).T),
            "gamma": gamma,
            "beta": beta,
        })
    return in_maps


def kernel(**inputs) -> np.ndarray:
    global _NC_CACHE
    in_maps = _make_in_maps(inputs)
    if _NC_CACHE is None:
        _NC_CACHE = build_program()
    res = run_bass_kernel_spmd(_NC_CACHE, in_maps, list(range(N_CORES)))
    out = np.empty((QALL, D), dtype=np.float32)
    for c in range(N_CORES):
        yc = res.results[c]["y"]
        for g in range(NW):
            out[g * WROWS + c * RW:g * WROWS + (c + 1) * RW] = \
                yc[g * RW:(g + 1) * RW]
    return np.ascontiguousarray(out.reshape(J, Q, D))


# revision 12
# speedup vs baseline: 1.0615x; 1.0601x over previous
"""ChunkCrossAttention Trainium2 kernel.

Math (per reference):
  x = chunk_embeddings[0]                      # (S, L)
  k, v = split(x @ W_kv.T)                     # (S, D) each
  scores = einsum('jqd,sd->jqs', q, k) / sqrt(D), masked
  attn = softmax(scores, -1)
  out = (attn @ v) @ W_out.T + q  -> LayerNorm(gamma, beta)

Strategy (8 NeuronCores):
  - KV projection sharded over S: each core projects its own 512 keys
    (k^T, v^T in [d, s] layout straight out of the PE).
  - W_out folded into v: v' = v @ W_out.T, with two ones columns appended
    so the attention matmul also emits the softmax denominator.
  - Attention partials: every core computes exp(q_all . k_loc) @ v'_loc
    over its local keys for ALL 8192 query rows (softmax without
    max-subtraction, mask folded into the Exp bias).
  - Partials are exchanged in 4 pipelined WAVES (2,4,4,6 chunks of 512
    query rows): bf16 AllToAll (copy-speed, ~2x a ReduceScatter) + a
    local f32 vector reduction + LayerNorm epilogue, overlapped with
    the next wave's attention compute.  The first wave is small so the
    first collective (which carries a ~30us setup premium) launches as
    early as possible.  Core c owns rows wrow0_g + c*rw_g + [0,rw_g) of
    wave g (host reassembles).
  - Attention inner loop is software-pipelined (sc0,sc1,av0,sc2,av1,...)
    so the PE never waits on the scalar engine's Exp.
  - Input DMA is staged: the x/w stream owns HBM during the KV
    projection; qT arrives per-wave, probe-pinned into the pipeline so
    transfers can't race ahead and steal bandwidth (the tile scheduler
    reorders anything without a data dependency).
"""
import sys

sys.path.insert(0, "/opt/trn_rl_repo")

import numpy as np

import concourse.bacc as bacc
import concourse.mybir as mybir
import concourse.tile as tile
from concourse.bass_utils import run_bass_kernel_spmd

N_CORES = 8
J, Q, D = 64, 128, 256
S, L = 4096, 4096
S_LOC = S // N_CORES          # 512 keys per core
QALL = J * Q                  # 8192 query rows total
QR = QALL // N_CORES          # 1024 query rows per core (output shard)
NCHUNK = 16                   # q-chunks of 512 rows
NW = 4
WLEN = [2, 4, 4, 6]           # chunks per wave
WSTART = [0, 2, 6, 10]
RWv = [wl * 512 // N_CORES for wl in WLEN]      # per-core rows per wave
DP = D + 2                    # attention free dim: D outputs + denom + pad
LN_EPS = 1e-5
SCALE = 1.0 / np.sqrt(D)

F32 = mybir.dt.float32
BF16 = mybir.dt.bfloat16
AF = mybir.ActivationFunctionType
ALU = mybir.AluOpType


def build_program():
    nc = bacc.Bacc(None, num_devices=N_CORES)

    xT = nc.declare_dram_parameter("xT", [L, S_LOC], BF16, isOutput=False)
    wkvT = nc.declare_dram_parameter("wkvT", [L, 2 * D], BF16, isOutput=False)
    qT = nc.declare_dram_parameter("qT", [D, QALL], BF16, isOutput=False)
    qres = nc.declare_dram_parameter("qres", [QR, D], F32, isOutput=False)
    woutT = nc.declare_dram_parameter("woutT", [D, D], BF16, isOutput=False)
    maskb = nc.declare_dram_parameter("maskb", [128, S_LOC // 128], F32,
                                      isOutput=False)
    gamma = nc.declare_dram_parameter("gamma", [D], F32, isOutput=False)
    beta = nc.declare_dram_parameter("beta", [D], F32, isOutput=False)
    y = nc.declare_dram_parameter("y", [QR, D], F32, isOutput=True)

    # per-wave partial exchange buffers:
    # a2a_in[g] slot c = bf16 partials for rows wrow0_g + c*rw_g + [0,rw_g)
    # after AllToAll, a2a_out[g] slot j = core j's partials for OUR rows.
    a2a_in = [nc.dram_tensor(f"a2a_in{g}", [N_CORES, RWv[g], DP], BF16)
              for g in range(NW)]
    a2a_out = [nc.dram_tensor(f"a2a_out{g}", [N_CORES, RWv[g], DP], BF16)
               for g in range(NW)]

    import concourse.bass as bass

    with tile.TileContext(nc) as tc:
        with tc.tile_pool(name="singles", bufs=1) as singles, \
             tc.tile_pool(name="xw", bufs=4) as xw, \
             tc.tile_pool(name="kv", bufs=1) as kvp, \
             tc.tile_pool(name="exp", bufs=4) as epool, \
             tc.tile_pool(name="part", bufs=3) as ppool, \
             tc.tile_pool(name="wave", bufs=2) as wpool, \
             tc.tile_pool(name="small", bufs=8) as small:

            # ---- small constants first (gpsimd queue) ----
            woutT_sb = singles.tile([128, 2, D], BF16)
            nc.gpsimd.dma_start(out=woutT_sb,
                                in_=woutT.rearrange("(dc p) d2 -> p dc d2",
                                                    p=128))
            maskb_sb = singles.tile([128, S_LOC // 128], F32)
            nc.gpsimd.dma_start(out=maskb_sb, in_=maskb[:, :])
            g_ap = gamma[:]
            gamma_sb = singles.tile([128, D], F32)
            nc.gpsimd.dma_start(out=gamma_sb, in_=bass.AP(
                tensor=g_ap.tensor, offset=g_ap.offset,
                ap=[[0, 128], g_ap.ap[0]]))
            b_ap = beta[:]
            beta_sb = singles.tile([128, D], F32)
            nc.gpsimd.dma_start(out=beta_sb, in_=bass.AP(
                tensor=b_ap.tensor, offset=b_ap.offset,
                ap=[[0, 128], b_ap.ap[0]]))
            eps_sb = singles.tile([128, 1], F32)
            nc.vector.memset(eps_sb, LN_EPS)

            qT_sb = singles.tile([128, 2, NCHUNK, 512], BF16)
            qres_sb = singles.tile([128, QR // 128, D], F32)

            def load_qT_wave(g):
                nc.gpsimd.dma_start(
                    out=qT_sb[:, :, WSTART[g]:WSTART[g] + WLEN[g], :],
                    in_=qT[:, WSTART[g] * 512:
                           (WSTART[g] + WLEN[g]) * 512].rearrange(
                        "(dc p) q -> p dc q", p=128))

            # ---- phase 1: local K^T / V^T projection over the S shard ----
            # x/w stream: xT on the sync queue, wkvT on the scalar queue;
            # first chunk small so the PE starts as early as possible.
            ps1 = tc.tile_pool(name="ps_kv", bufs=1, space="PSUM")
            ps_kv = ps1.__enter__()
            acc = [ps_kv.tile([128, S_LOC], F32, tag=f"acc{h}", name=f"acc{h}")
                   for h in range(4)]
            chunks = [(0, 128), (128, 384)] + [(512 * i, 512)
                                               for i in range(1, 8)]
            n_mm = sum(nr // 128 for _, nr in chunks) * 4
            mm_i = 0
            xt_probe = None
            for ci, (r0, nr) in enumerate(chunks):
                na = nr // 128
                xt = xw.tile([128, na, S_LOC], BF16, tag=f"xt{na}")
                nc.sync.dma_start(
                    out=xt,
                    in_=xT[r0:r0 + nr, :].rearrange("(a p) s -> p a s", p=128))
                wt = xw.tile([128, na, 2 * D], BF16, tag=f"wt{na}")
                nc.scalar.dma_start(
                    out=wt,
                    in_=wkvT[r0:r0 + nr, :].rearrange("(a p) s -> p a s",
                                                      p=128))
                for a in range(na):
                    for h in range(4):
                        nc.tensor.matmul(acc[h], wt[:, a, h * 128:(h + 1) * 128],
                                         xt[:, a, :], start=(mm_i == 0),
                                         stop=(mm_i == n_mm - 4 + h))
                    mm_i += 4
                if ci == 3:
                    # release qT wave 0 only once the x/w stream is mostly
                    # through: RAW on this x-chunk pins the probe, WAR on
                    # qT_sb pins the transfer behind it
                    probe = small.tile([128, 1], BF16, tag="probe")
                    nc.vector.tensor_add(out=probe, in0=qT_sb[:, 0, 0, 0:1],
                                         in1=xt[:, 0, 0:1])
                    load_qT_wave(0)

            kT_loc = kvp.tile([128, 2, S_LOC], BF16)
            nc.scalar.copy(out=kT_loc[:, 0, :], in_=acc[0])
            nc.scalar.copy(out=kT_loc[:, 1, :], in_=acc[1])
            vT_loc = kvp.tile([128, 2, S_LOC], BF16)
            nc.scalar.copy(out=vT_loc[:, 0, :], in_=acc[2])
            nc.scalar.copy(out=vT_loc[:, 1, :], in_=acc[3])

            # ---- v' = v @ W_out.T, plus ones columns -> [s, DP] ----
            vp_sb = kvp.tile([128, 4, DP], BF16)
            nc.vector.memset(vp_sb, 1.0)
            for ss in range(4):
                pv = ps_kv.tile([128, D], F32, tag="pv", name="pv")
                for dc in range(2):
                    nc.tensor.matmul(
                        pv, vT_loc[:, dc, ss * 128:(ss + 1) * 128],
                        woutT_sb[:, dc, :], start=(dc == 0), stop=(dc == 1))
                nc.vector.tensor_copy(out=vp_sb[:, ss, 0:D], in_=pv)
            ps1.__exit__(None, None, None)

            # ---- phase 2: partial attention, 16 chunks in 4 waves ----
            ps3 = tc.tile_pool(name="ps_at", bufs=1, space="PSUM")
            ps_at = ps3.__enter__()
            ps3b = tc.tile_pool(name="ps_sc", bufs=3, space="PSUM")
            ps_sc = ps3b.__enter__()

            n_st = S_LOC // 128                       # 4 local key tiles

            def scores(p, st):
                sc = ps_sc.tile([128, 512], F32, tag="sc")
                for dc in range(2):
                    nc.tensor.matmul(
                        sc, kT_loc[:, dc, st * 128:(st + 1) * 128],
                        qT_sb[:, dc, p, :], start=(dc == 0), stop=(dc == 1))
                ex = epool.tile([128, 512], BF16, tag="ex")
                nc.scalar.activation(out=ex, in_=sc, func=AF.Exp,
                                     bias=maskb_sb[:, st:st + 1], scale=SCALE)
                return ex

            for g in range(NW):
                rw = RWv[g]
                a2a_r = a2a_in[g].rearrange("s (t p) f -> s p t f", p=128)
                for p in range(WSTART[g], WSTART[g] + WLEN[g]):
                    at = [ps_at.tile([128, DP], F32, tag=f"at{i}",
                                     name=f"at{i}") for i in range(4)]

                    ex = [None] * n_st

                    def av(st):
                        for qt in range(4):
                            nc.tensor.matmul(
                                at[qt], ex[st][:, qt * 128:(qt + 1) * 128],
                                vp_sb[:, st, :],
                                start=(st == 0), stop=(st == n_st - 1))

                    # software pipeline: sc0,sc1,av0,sc2,av1,sc3,av2,av3
                    ex[0] = scores(p, 0)
                    ex[1] = scores(p, 1)
                    av(0)
                    ex[2] = scores(p, 2)
                    av(1)
                    ex[3] = scores(p, 3)
                    av(2)
                    av(3)

                    part = ppool.tile([128, 4, DP], BF16, tag="part")
                    for qt in range(4):
                        nc.vector.tensor_copy(out=part[:, qt, :], in_=at[qt])
                    # group contiguous qt-tiles by target slot
                    o0 = (p - WSTART[g]) * 512
                    qt = 0
                    while qt < 4:
                        slot, j0 = divmod(o0 + qt * 128, rw)
                        nqt = min(4 - qt, (rw - j0) // 128)
                        t0 = j0 // 128
                        nc.scalar.dma_start(
                            out=a2a_r[slot][:, t0:t0 + nqt, :],
                            in_=part[:, qt:qt + nqt, :])
                        qt += nqt

                    # probe-pinned late loads (see module docstring)
                    if p in (0, 1, 2):
                        if p == 2:
                            qprobe = small.tile([128, 1], F32, tag="qprobe")
                            nc.vector.tensor_add(out=qprobe,
                                                 in0=qres_sb[:, 0, 0:1],
                                                 in1=part[:, 0, 0:1])
                            nc.gpsimd.dma_start(
                                out=qres_sb,
                                in_=qres.rearrange("(t p) d -> p t d", p=128))
                        elif p == 0:
                            probe = small.tile([128, 1], BF16, tag="probe")
                            nc.vector.tensor_add(
                                out=probe,
                                in0=qT_sb[:, 0, WSTART[1], 0:1],
                                in1=part[:, 0, 0:1])
                            load_qT_wave(1)
                        else:
                            for gl in (2, 3):
                                probe = small.tile([128, 1], BF16,
                                                   tag="probe")
                                nc.vector.tensor_add(
                                    out=probe,
                                    in0=qT_sb[:, 0, WSTART[gl], 0:1],
                                    in1=part[:, 0, 0:1])
                                load_qT_wave(gl)

                nc.gpsimd.collective_compute(
                    "AllToAll", ALU.bypass,
                    replica_groups=[list(range(N_CORES))],
                    ins=[a2a_in[g][:, :, :]], outs=[a2a_out[g][:, :, :]])

            ps3b.__exit__(None, None, None)
            ps3.__exit__(None, None, None)

            # ---- phase 3: per-wave local reduce + epilogue ----
            yrow0 = [sum(RWv[:g]) for g in range(NW)]
            for g in range(NW):
                rw = RWv[g]
                nt = rw // 128
                ao = wpool.tile([128, N_CORES * nt, DP], BF16, tag=f"ao{nt}")
                nc.gpsimd.dma_start(
                    out=ao,
                    in_=a2a_out[g].rearrange("s (t p) f -> p (s t) f", p=128))
                red = wpool.tile([128, nt, DP], F32, tag=f"red{nt}")
                ao_v = ao.rearrange("p (s t) f -> p t f s", s=N_CORES)
                nc.vector.tensor_reduce(
                    out=red.rearrange("p t f -> p (t f)"), in_=ao_v,
                    axis=mybir.AxisListType.X, op=ALU.add)
                h_half = wpool.tile([128, nt, D], F32, tag=f"h{nt}")
                for t in range(nt):
                    hs = h_half[:, t, :]
                    rec = small.tile([128, 1], F32, tag="rec")
                    nc.vector.reciprocal(out=rec, in_=red[:, t, D:D + 1])
                    nc.vector.tensor_scalar_mul(out=hs, in0=red[:, t, 0:D],
                                                scalar1=rec)
                    nc.vector.tensor_add(
                        out=hs, in0=hs,
                        in1=qres_sb[:, yrow0[g] // 128 + t, :])
                    stats = small.tile([128, 6], F32, tag="stats")
                    nc.vector.bn_stats(out=stats, in_=hs)
                    mv = small.tile([128, 2], F32, tag="mv")
                    nc.vector.bn_aggr(out=mv, in_=stats)
                    rstd = small.tile([128, 1], F32, tag="rstd")
                    nc.scalar.activation(out=rstd, in_=mv[:, 1:2], func=AF.Sqrt,
                                         bias=eps_sb, scale=1.0)
                    nc.vector.reciprocal(out=rstd, in_=rstd)
                    nc.vector.tensor_scalar(out=hs, in0=hs,
                                            scalar1=mv[:, 0:1], scalar2=rstd,
                                            op0=ALU.subtract, op1=ALU.mult)
                    nc.vector.tensor_mul(out=hs, in0=hs, in1=gamma_sb)
                    nc.vector.tensor_add(out=hs, in0=hs, in1=beta_sb)
                nc.gpsimd.dma_start(
                    out=y[yrow0[g]:yrow0[g] + rw, :].rearrange(
                        "(t p) d -> p t d", p=128),
                    in_=h_half)

    nc.finalize()
    return nc


_NC_CACHE = None


def _make_in_maps(inputs):
    jq = np.asarray(inputs["justice_queries"], dtype=np.float32)
    x = np.asarray(inputs["chunk_embeddings"], dtype=np.float32)[0]
    mask = np.asarray(inputs["chunk_mask"])
    wkv = np.asarray(inputs["W_kv"], dtype=np.float32)
    wout = np.asarray(inputs["W_out"], dtype=np.float32)
    gamma = np.asarray(inputs["ln_gamma"], dtype=np.float32)
    beta = np.asarray(inputs["ln_beta"], dtype=np.float32)

    import ml_dtypes
    bf16 = ml_dtypes.bfloat16
    xT = np.ascontiguousarray(x.T.astype(bf16))         # (L, S)
    wkvT = np.ascontiguousarray(wkv.T.astype(bf16))     # (L, 2D)
    flat = np.ascontiguousarray(jq.reshape(J * Q, D))   # (8192, D)
    qT = np.ascontiguousarray(flat.T.astype(bf16))      # (D, 8192)
    woutT = np.ascontiguousarray(wout.T.astype(bf16))   # (D, D)
    mb_full = np.where(mask != 0, 0.0, -1e30).astype(np.float32)

    in_maps = []
    for c in range(N_CORES):
        mb = mb_full[c * S_LOC:(c + 1) * S_LOC]
        # core c owns rows wrow0_g + c*rw_g + [0,rw_g) of wave g
        rows = np.concatenate([
            np.arange(WSTART[g] * 512 + c * RWv[g],
                      WSTART[g] * 512 + (c + 1) * RWv[g])
            for g in range(NW)])
        in_maps.append({
            "xT": np.ascontiguousarray(xT[:, c * S_LOC:(c + 1) * S_LOC]),
            "wkvT": wkvT,
            "qT": qT,
            "qres": np.ascontiguousarray(flat[rows, :]),
            "woutT": woutT,
            "maskb": np.ascontiguousarray(mb.reshape(S_LOC // 128, 128).T),
            "gamma": gamma,
            "beta": beta,
        })
    return in_maps


def kernel(**inputs) -> np.ndarray:
    global _NC_CACHE
    in_maps = _make_in_maps(inputs)
    if _NC_CACHE is None:
        _NC_CACHE = build_program()
    res = run_bass_kernel_spmd(_NC_CACHE, in_maps, list(range(N_CORES)))
    out = np.empty((QALL, D), dtype=np.float32)
    for c in range(N_CORES):
        yc = res.results[c]["y"]
        r = 0
        for g in range(NW):
            rw = RWv[g]
            out[WSTART[g] * 512 + c * rw:WSTART[g] * 512 + (c + 1) * rw] = \
                yc[r:r + rw]
            r += rw
    return np.ascontiguousarray(out.reshape(J, Q, D))


# revision 13
# speedup vs baseline: 1.2190x; 1.1484x over previous
"""ChunkCrossAttention Trainium2 kernel.

Math (per reference):
  x = chunk_embeddings[0]                      # (S, L)
  k, v = split(x @ W_kv.T)                     # (S, D) each
  scores = einsum('jqd,sd->jqs', q, k) / sqrt(D), masked
  attn = softmax(scores, -1)
  out = (attn @ v) @ W_out.T + q  -> LayerNorm(gamma, beta)

Strategy (8 NeuronCores) — AllGather-KV:
  - KV projection sharded over S: each core projects its own 512 keys
    (k^T, v^T in [d, s] layout straight out of the PE).
  - W_out folded into v: v' = v @ W_out.T, with two ones columns
    appended so the attention matmul also emits the softmax denominator.
  - Each core writes its K^T / v' block (526 KB bf16) to DRAM and ONE
    AllGather (Shared pair-HBM output = the fast collective path)
    replicates all 4096 keys everywhere.  This is the only collective:
    partial-sum exchange, cross-core reduction and their per-collective
    latency floors are gone entirely.
  - Each core then attends its own 1024 query rows over all 4096 keys
    (same FLOPs as the key-sharded variant), so softmax normalization,
    residual and LayerNorm are purely local.  Gathered K/V blocks are
    DMA'd to SBUF per key-block, so attention starts as soon as the
    first block lands.
  - Softmax runs without max-subtraction (scores ~ N(0,1), exp is safe
    in f32), mask folded into the Exp bias.
  - Attention inner loop is software-pipelined (sc_i then av_{i-1}) so
    the PE never waits on the scalar engine's Exp.
"""
import sys

sys.path.insert(0, "/opt/trn_rl_repo")

import numpy as np

import concourse.bacc as bacc
import concourse.mybir as mybir
import concourse.tile as tile
from concourse.bass_utils import run_bass_kernel_spmd

N_CORES = 8
J, Q, D = 64, 128, 256
S, L = 4096, 4096
S_LOC = S // N_CORES          # 512 keys per core
QALL = J * Q                  # 8192 query rows total
QR = QALL // N_CORES          # 1024 query rows per core (output shard)
DP = D + 2                    # attention free dim: D outputs + denom + pad
KELEM = 2 * 128 * 512         # K^T elems in the kv blob
VELEM = 4 * 128 * DP          # v' elems in the kv blob
LN_EPS = 1e-5
SCALE = 1.0 / np.sqrt(D)

F32 = mybir.dt.float32
BF16 = mybir.dt.bfloat16
AF = mybir.ActivationFunctionType
ALU = mybir.AluOpType


def build_program():
    nc = bacc.Bacc(None, num_devices=N_CORES)

    xT = nc.declare_dram_parameter("xT", [L, S_LOC], BF16, isOutput=False)
    wkvT = nc.declare_dram_parameter("wkvT", [L, 2 * D], BF16, isOutput=False)
    qT = nc.declare_dram_parameter("qT", [D, QR], BF16, isOutput=False)
    qres = nc.declare_dram_parameter("qres", [QR, D], F32, isOutput=False)
    woutT = nc.declare_dram_parameter("woutT", [D, D], BF16, isOutput=False)
    maskb = nc.declare_dram_parameter("maskb", [128, S // 128], F32,
                                      isOutput=False)
    gamma = nc.declare_dram_parameter("gamma", [D], F32, isOutput=False)
    beta = nc.declare_dram_parameter("beta", [D], F32, isOutput=False)
    y = nc.declare_dram_parameter("y", [QR, D], F32, isOutput=True)

    kv_loc = nc.dram_tensor("kv_loc", [KELEM + VELEM], BF16)
    kv_sh = nc.dram_tensor("kv_sh", [N_CORES, KELEM + VELEM], BF16,
                           addr_space="Shared")

    import concourse.bass as bass

    with tile.TileContext(nc) as tc:
        with tc.tile_pool(name="singles", bufs=1) as singles, \
             tc.tile_pool(name="xw", bufs=4) as xw, \
             tc.tile_pool(name="kv", bufs=1) as kvp, \
             tc.tile_pool(name="exp", bufs=4) as epool, \
             tc.tile_pool(name="hpool", bufs=2) as hpool, \
             tc.tile_pool(name="small", bufs=8) as small:

            # ---- constants + per-core q inputs (gpsimd queue) ----
            woutT_sb = singles.tile([128, 2, D], BF16)
            nc.gpsimd.dma_start(out=woutT_sb,
                                in_=woutT.rearrange("(dc p) d2 -> p dc d2",
                                                    p=128))
            maskb_sb = singles.tile([128, S // 128], F32)
            nc.gpsimd.dma_start(out=maskb_sb, in_=maskb[:, :])
            g_ap = gamma[:]
            gamma_sb = singles.tile([128, D], F32)
            nc.gpsimd.dma_start(out=gamma_sb, in_=bass.AP(
                tensor=g_ap.tensor, offset=g_ap.offset,
                ap=[[0, 128], g_ap.ap[0]]))
            b_ap = beta[:]
            beta_sb = singles.tile([128, D], F32)
            nc.gpsimd.dma_start(out=beta_sb, in_=bass.AP(
                tensor=b_ap.tensor, offset=b_ap.offset,
                ap=[[0, 128], b_ap.ap[0]]))
            eps_sb = singles.tile([128, 1], F32)
            nc.vector.memset(eps_sb, LN_EPS)
            qT_sb = singles.tile([128, 2, QR], BF16)
            nc.gpsimd.dma_start(out=qT_sb,
                                in_=qT.rearrange("(dc p) q -> p dc q", p=128))
            qres_sb = singles.tile([128, QR // 128, D], F32)
            nc.gpsimd.dma_start(out=qres_sb,
                                in_=qres.rearrange("(t p) d -> p t d", p=128))

            # ---- phase 1: local K^T / V^T projection over the S shard ----
            # x on the sync queue, w on the scalar queue; first chunk small
            # so the PE starts as early as possible.
            ps1 = tc.tile_pool(name="ps_kv", bufs=1, space="PSUM")
            ps_kv = ps1.__enter__()
            acc = [ps_kv.tile([128, S_LOC], F32, tag=f"acc{h}", name=f"acc{h}")
                   for h in range(4)]
            chunks = [(0, 128), (128, 384)] + [(512 * i, 512)
                                               for i in range(1, 8)]
            n_mm = sum(nr // 128 for _, nr in chunks) * 4
            mm_i = 0
            for (r0, nr) in chunks:
                na = nr // 128
                xt = xw.tile([128, na, S_LOC], BF16, tag=f"xt{na}")
                nc.sync.dma_start(
                    out=xt,
                    in_=xT[r0:r0 + nr, :].rearrange("(a p) s -> p a s", p=128))
                wt = xw.tile([128, na, 2 * D], BF16, tag=f"wt{na}")
                nc.scalar.dma_start(
                    out=wt,
                    in_=wkvT[r0:r0 + nr, :].rearrange("(a p) s -> p a s",
                                                      p=128))
                for a in range(na):
                    for h in range(4):
                        nc.tensor.matmul(acc[h], wt[:, a, h * 128:(h + 1) * 128],
                                         xt[:, a, :], start=(mm_i == 0),
                                         stop=(mm_i == n_mm - 4 + h))
                    mm_i += 4

            kT_loc = kvp.tile([128, 2, S_LOC], BF16)
            nc.scalar.copy(out=kT_loc[:, 0, :], in_=acc[0])
            nc.scalar.copy(out=kT_loc[:, 1, :], in_=acc[1])
            vT_loc = kvp.tile([128, 2, S_LOC], BF16)
            nc.scalar.copy(out=vT_loc[:, 0, :], in_=acc[2])
            nc.scalar.copy(out=vT_loc[:, 1, :], in_=acc[3])

            # ---- v' = v @ W_out.T, plus ones columns -> [s, DP] ----
            vp_sb = kvp.tile([128, 4, DP], BF16)
            nc.vector.memset(vp_sb, 1.0)
            for ss in range(4):
                pv = ps_kv.tile([128, D], F32, tag="pv", name="pv")
                for dc in range(2):
                    nc.tensor.matmul(
                        pv, vT_loc[:, dc, ss * 128:(ss + 1) * 128],
                        woutT_sb[:, dc, :], start=(dc == 0), stop=(dc == 1))
                nc.vector.tensor_copy(out=vp_sb[:, ss, 0:D], in_=pv)
            ps1.__exit__(None, None, None)

            # ---- publish local K/v' and AllGather all 4096 keys ----
            nc.sync.dma_start(
                out=kv_loc[0:KELEM].rearrange("(dc p s) -> p dc s",
                                              p=128, s=512),
                in_=kT_loc)
            nc.sync.dma_start(
                out=kv_loc[KELEM:KELEM + VELEM].rearrange(
                    "(ss p f) -> p ss f", p=128, f=DP),
                in_=vp_sb)
            nc.gpsimd.collective_compute(
                "AllGather", ALU.bypass,
                replica_groups=[list(range(N_CORES))],
                ins=[kv_loc[:]], outs=[kv_sh[:, :]])

            # gathered K/v' -> SBUF, one DMA pair per key-block so the
            # attention pipeline starts on block 0 immediately
            kT_all = kvp.tile([128, N_CORES, 2, 512], BF16)
            vp_all = kvp.tile([128, N_CORES, 4, DP], BF16)
            for r in range(N_CORES):
                nc.sync.dma_start(
                    out=kT_all[:, r, :, :],
                    in_=kv_sh[r, 0:KELEM].rearrange("(dc p s) -> p dc s",
                                                    p=128, s=512))
                nc.gpsimd.dma_start(
                    out=vp_all[:, r, :, :],
                    in_=kv_sh[r, KELEM:KELEM + VELEM].rearrange(
                        "(ss p f) -> p ss f", p=128, f=DP))

            # ---- phase 2: attention for our 1024 rows over all keys ----
            ps3 = tc.tile_pool(name="ps_at", bufs=1, space="PSUM")
            ps_at = ps3.__enter__()
            ps3b = tc.tile_pool(name="ps_sc", bufs=3, space="PSUM")
            ps_sc = ps3b.__enter__()

            NST = S // 128                            # 32 key tiles

            def scores(p, i):
                blk, st = i // 4, i % 4
                sc = ps_sc.tile([128, 512], F32, tag="sc")
                for dc in range(2):
                    nc.tensor.matmul(
                        sc, kT_all[:, blk, dc, st * 128:(st + 1) * 128],
                        qT_sb[:, dc, p * 512:(p + 1) * 512],
                        start=(dc == 0), stop=(dc == 1))
                ex = epool.tile([128, 512], BF16, tag="ex")
                nc.scalar.activation(out=ex, in_=sc, func=AF.Exp,
                                     bias=maskb_sb[:, i:i + 1], scale=SCALE)
                return ex

            y_r = y.rearrange("(c t p) d -> c p t d", c=2, p=128)
            for p in range(2):                        # q chunks of 512 rows
                at = [ps_at.tile([128, DP], F32, tag=f"at{i}", name=f"at{i}")
                      for i in range(4)]
                ex = [None] * NST

                def av(i):
                    blk, st = i // 4, i % 4
                    for qt in range(4):
                        nc.tensor.matmul(
                            at[qt], ex[i][:, qt * 128:(qt + 1) * 128],
                            vp_all[:, blk, st, :],
                            start=(i == 0), stop=(i == NST - 1))

                ex[0] = scores(p, 0)
                for i in range(1, NST):
                    ex[i] = scores(p, i)
                    av(i - 1)
                av(NST - 1)

                # ---- epilogue straight out of PSUM: normalize, residual,
                # LayerNorm ----
                h_half = hpool.tile([128, 4, D], F32, tag="h")
                for qt in range(4):
                    hs = h_half[:, qt, :]
                    rec = small.tile([128, 1], F32, tag="rec")
                    nc.vector.reciprocal(out=rec, in_=at[qt][:, D:D + 1])
                    nc.vector.tensor_scalar_mul(out=hs, in0=at[qt][:, 0:D],
                                                scalar1=rec)
                    nc.vector.tensor_add(out=hs, in0=hs,
                                         in1=qres_sb[:, 4 * p + qt, :])
                    stats = small.tile([128, 6], F32, tag="stats")
                    nc.vector.bn_stats(out=stats, in_=hs)
                    mv = small.tile([128, 2], F32, tag="mv")
                    nc.vector.bn_aggr(out=mv, in_=stats)
                    rstd = small.tile([128, 1], F32, tag="rstd")
                    nc.scalar.activation(out=rstd, in_=mv[:, 1:2], func=AF.Sqrt,
                                         bias=eps_sb, scale=1.0)
                    nc.vector.reciprocal(out=rstd, in_=rstd)
                    nc.vector.tensor_scalar(out=hs, in0=hs,
                                            scalar1=mv[:, 0:1], scalar2=rstd,
                                            op0=ALU.subtract, op1=ALU.mult)
                    nc.vector.tensor_mul(out=hs, in0=hs, in1=gamma_sb)
                    nc.vector.tensor_add(out=hs, in0=hs, in1=beta_sb)
                nc.gpsimd.dma_start(out=y_r[p], in_=h_half)

            ps3b.__exit__(None, None, None)
            ps3.__exit__(None, None, None)

    nc.finalize()
    return nc


_NC_CACHE = None


def _make_in_maps(inputs):
    jq = np.asarray(inputs["justice_queries"], dtype=np.float32)
    x = np.asarray(inputs["chunk_embeddings"], dtype=np.float32)[0]
    mask = np.asarray(inputs["chunk_mask"])
    wkv = np.asarray(inputs["W_kv"], dtype=np.float32)
    wout = np.asarray(inputs["W_out"], dtype=np.float32)
    gamma = np.asarray(inputs["ln_gamma"], dtype=np.float32)
    beta = np.asarray(inputs["ln_beta"], dtype=np.float32)

    import ml_dtypes
    bf16 = ml_dtypes.bfloat16
    xT = np.ascontiguousarray(x.T.astype(bf16))         # (L, S)
    wkvT = np.ascontiguousarray(wkv.T.astype(bf16))     # (L, 2D)
    flat = np.ascontiguousarray(jq.reshape(J * Q, D))   # (8192, D)
    qT = np.ascontiguousarray(flat.T.astype(bf16))      # (D, 8192)
    woutT = np.ascontiguousarray(wout.T.astype(bf16))   # (D, D)
    mb_full = np.where(mask != 0, 0.0, -1e30).astype(np.float32)
    mb = np.ascontiguousarray(mb_full.reshape(S // 128, 128).T)

    in_maps = []
    for c in range(N_CORES):
        in_maps.append({
            "xT": np.ascontiguousarray(xT[:, c * S_LOC:(c + 1) * S_LOC]),
            "wkvT": wkvT,
            "qT": np.ascontiguousarray(qT[:, c * QR:(c + 1) * QR]),
            "qres": np.ascontiguousarray(flat[c * QR:(c + 1) * QR, :]),
            "woutT": woutT,
            "maskb": mb,
            "gamma": gamma,
            "beta": beta,
        })
    return in_maps


def kernel(**inputs) -> np.ndarray:
    global _NC_CACHE
    in_maps = _make_in_maps(inputs)
    if _NC_CACHE is None:
        _NC_CACHE = build_program()
    res = run_bass_kernel_spmd(_NC_CACHE, in_maps, list(range(N_CORES)))
    out = np.concatenate([res.results[c]["y"] for c in range(N_CORES)], axis=0)
    return np.ascontiguousarray(out.reshape(J, Q, D).astype(np.float32))


# revision 17
# speedup vs baseline: 1.2198x; 1.0007x over previous
"""ChunkCrossAttention Trainium2 kernel.

Math (per reference):
  x = chunk_embeddings[0]                      # (S, L)
  k, v = split(x @ W_kv.T)                     # (S, D) each
  scores = einsum('jqd,sd->jqs', q, k) / sqrt(D), masked
  attn = softmax(scores, -1)
  out = (attn @ v) @ W_out.T + q  -> LayerNorm(gamma, beta)

Strategy (8 NeuronCores) — AllGather-KV:
  - KV projection sharded over S: each core projects its own 512 keys
    (k^T, v^T in [d, s] layout straight out of the PE).
  - W_out folded into v: v' = v @ W_out.T, with two ones columns
    appended so the attention matmul also emits the softmax denominator.
  - Each core writes its K^T / v' block (526 KB bf16) to DRAM and ONE
    AllGather (Shared pair-HBM output = the fast collective path)
    replicates all 4096 keys everywhere.  This is the only collective:
    partial-sum exchange, cross-core reduction and their per-collective
    latency floors are gone entirely.
  - Each core then attends its own 1024 query rows over all 4096 keys
    (same FLOPs as the key-sharded variant), so softmax normalization,
    residual and LayerNorm are purely local.  Gathered K/V blocks are
    DMA'd to SBUF per key-block, so attention starts as soon as the
    first block lands.
  - Softmax runs without max-subtraction (scores ~ N(0,1), exp is safe
    in f32), mask folded into the Exp bias.
  - Attention inner loop is software-pipelined (sc_i then av_{i-1}) so
    the PE never waits on the scalar engine's Exp.
"""
import sys

sys.path.insert(0, "/opt/trn_rl_repo")

import numpy as np

import concourse.bacc as bacc
import concourse.mybir as mybir
import concourse.tile as tile
from concourse.bass_utils import run_bass_kernel_spmd

N_CORES = 8
J, Q, D = 64, 128, 256
S, L = 4096, 4096
S_LOC = S // N_CORES          # 512 keys per core
QALL = J * Q                  # 8192 query rows total
QR = QALL // N_CORES          # 1024 query rows per core (output shard)
DP = D + 2                    # attention free dim: D outputs + denom + pad
KELEM = 2 * 128 * 512         # K^T elems in the kv blob
VELEM = 4 * 128 * DP          # v' elems in the kv blob
LN_EPS = 1e-5
SCALE = 1.0 / np.sqrt(D)

F32 = mybir.dt.float32
BF16 = mybir.dt.bfloat16
AF = mybir.ActivationFunctionType
ALU = mybir.AluOpType


def build_program():
    nc = bacc.Bacc(None, num_devices=N_CORES)

    xT = nc.declare_dram_parameter("xT", [L, S_LOC], BF16, isOutput=False)
    wkvT = nc.declare_dram_parameter("wkvT", [L, 2 * D], BF16, isOutput=False)
    qT = nc.declare_dram_parameter("qT", [D, QR], BF16, isOutput=False)
    qres = nc.declare_dram_parameter("qres", [QR, D], F32, isOutput=False)
    woutT = nc.declare_dram_parameter("woutT", [D, D], BF16, isOutput=False)
    maskb = nc.declare_dram_parameter("maskb", [128, S // 128], F32,
                                      isOutput=False)
    gamma = nc.declare_dram_parameter("gamma", [D], F32, isOutput=False)
    beta = nc.declare_dram_parameter("beta", [D], F32, isOutput=False)
    y = nc.declare_dram_parameter("y", [QR, D], F32, isOutput=True)

    k_loc = nc.dram_tensor("k_loc", [KELEM], BF16)
    k_sh = nc.dram_tensor("k_sh", [N_CORES, KELEM], BF16,
                          addr_space="Shared")
    v_loc = nc.dram_tensor("v_loc", [VELEM], BF16)
    v_sh = nc.dram_tensor("v_sh", [N_CORES, VELEM], BF16,
                          addr_space="Shared")
    # tiny warmup AllGather: its doorbell rings at ~13us so the collective
    # stream's one-time setup cost burns during phase 1, not after it
    warm_loc = nc.dram_tensor("warm_loc", [128], BF16)
    warm_sh = nc.dram_tensor("warm_sh", [N_CORES, 128], BF16,
                             addr_space="Shared")

    import concourse.bass as bass

    with tile.TileContext(nc) as tc:
        with tc.tile_pool(name="singles", bufs=1) as singles, \
             tc.tile_pool(name="xw", bufs=4) as xw, \
             tc.tile_pool(name="kv", bufs=1) as kvp, \
             tc.tile_pool(name="exp", bufs=4) as epool, \
             tc.tile_pool(name="hpool", bufs=2) as hpool, \
             tc.tile_pool(name="small", bufs=8) as small:

            # ---- constants + per-core q inputs (gpsimd queue) ----
            woutT_sb = singles.tile([128, 2, D], BF16)
            nc.gpsimd.dma_start(out=woutT_sb,
                                in_=woutT.rearrange("(dc p) d2 -> p dc d2",
                                                    p=128))
            maskb_sb = singles.tile([128, S // 128], F32)
            nc.gpsimd.dma_start(out=maskb_sb, in_=maskb[:, :])
            g_ap = gamma[:]
            gamma_sb = singles.tile([128, D], F32)
            nc.gpsimd.dma_start(out=gamma_sb, in_=bass.AP(
                tensor=g_ap.tensor, offset=g_ap.offset,
                ap=[[0, 128], g_ap.ap[0]]))
            b_ap = beta[:]
            beta_sb = singles.tile([128, D], F32)
            nc.gpsimd.dma_start(out=beta_sb, in_=bass.AP(
                tensor=b_ap.tensor, offset=b_ap.offset,
                ap=[[0, 128], b_ap.ap[0]]))
            eps_sb = singles.tile([128, 1], F32)
            nc.vector.memset(eps_sb, LN_EPS)
            warm_sb = small.tile([128, 128], BF16, tag="warm")
            nc.vector.memset(warm_sb, 0.0)
            nc.sync.dma_start(out=warm_loc[:], in_=warm_sb[0:1, :])
            nc.gpsimd.collective_compute(
                "AllGather", ALU.bypass,
                replica_groups=[list(range(N_CORES))],
                ins=[warm_loc[:]], outs=[warm_sh[:, :]])
            qT_sb = singles.tile([128, 2, QR], BF16)
            nc.gpsimd.dma_start(out=qT_sb,
                                in_=qT.rearrange("(dc p) q -> p dc q", p=128))
            qres_sb = singles.tile([128, QR // 128, D], F32)
            nc.gpsimd.dma_start(out=qres_sb,
                                in_=qres.rearrange("(t p) d -> p t d", p=128))

            # ---- phase 1: local K^T / V^T projection over the S shard ----
            # x on the sync queue, w on the scalar queue; first chunk small
            # so the PE starts as early as possible.
            ps1 = tc.tile_pool(name="ps_kv", bufs=1, space="PSUM")
            ps_kv = ps1.__enter__()
            acc = [ps_kv.tile([128, S_LOC], F32, tag=f"acc{h}", name=f"acc{h}")
                   for h in range(4)]
            chunks = [(0, 128), (128, 384)] + [(512 * i, 512)
                                               for i in range(1, 8)]
            n_mm = sum(nr // 128 for _, nr in chunks) * 4
            mm_i = 0
            for (r0, nr) in chunks:
                na = nr // 128
                xt = xw.tile([128, na, S_LOC], BF16, tag=f"xt{na}")
                nc.sync.dma_start(
                    out=xt,
                    in_=xT[r0:r0 + nr, :].rearrange("(a p) s -> p a s", p=128))
                wt = xw.tile([128, na, 2 * D], BF16, tag=f"wt{na}")
                nc.scalar.dma_start(
                    out=wt,
                    in_=wkvT[r0:r0 + nr, :].rearrange("(a p) s -> p a s",
                                                      p=128))
                for a in range(na):
                    for h in range(4):
                        nc.tensor.matmul(acc[h], wt[:, a, h * 128:(h + 1) * 128],
                                         xt[:, a, :], start=(mm_i == 0),
                                         stop=(mm_i == n_mm - 4 + h))
                    mm_i += 4

            kT_loc = kvp.tile([128, 2, S_LOC], BF16)
            nc.scalar.copy(out=kT_loc[:, 0, :], in_=acc[0])
            nc.scalar.copy(out=kT_loc[:, 1, :], in_=acc[1])
            # publish + AllGather K immediately — scores only need K, so
            # this collective runs while v' is still being folded
            nc.sync.dma_start(
                out=k_loc[:].rearrange("(dc p s) -> p dc s", p=128, s=512),
                in_=kT_loc)
            nc.gpsimd.collective_compute(
                "AllGather", ALU.bypass,
                replica_groups=[list(range(N_CORES))],
                ins=[k_loc[:]], outs=[k_sh[:, :]])

            vT_loc = kvp.tile([128, 2, S_LOC], BF16)
            nc.scalar.copy(out=vT_loc[:, 0, :], in_=acc[2])
            nc.scalar.copy(out=vT_loc[:, 1, :], in_=acc[3])

            # ---- v' = v @ W_out.T, plus ones columns -> [s, DP] ----
            vp_sb = kvp.tile([128, 4, DP], BF16)
            nc.vector.memset(vp_sb, 1.0)
            for ss in range(4):
                pv = ps_kv.tile([128, D], F32, tag="pv", name="pv")
                for dc in range(2):
                    nc.tensor.matmul(
                        pv, vT_loc[:, dc, ss * 128:(ss + 1) * 128],
                        woutT_sb[:, dc, :], start=(dc == 0), stop=(dc == 1))
                nc.vector.tensor_copy(out=vp_sb[:, ss, 0:D], in_=pv)
            ps1.__exit__(None, None, None)

            nc.sync.dma_start(
                out=v_loc[:].rearrange("(ss p f) -> p ss f", p=128, f=DP),
                in_=vp_sb)
            nc.gpsimd.collective_compute(
                "AllGather", ALU.bypass,
                replica_groups=[list(range(N_CORES))],
                ins=[v_loc[:]], outs=[v_sh[:, :]])

            # gathered K/v' -> SBUF, one DMA pair per key-block so the
            # attention pipeline starts on block 0 immediately
            kT_all = kvp.tile([128, N_CORES, 2, 512], BF16)
            vp_all = kvp.tile([128, N_CORES, 4, DP], BF16)
            for r in range(N_CORES):
                nc.sync.dma_start(
                    out=kT_all[:, r, :, :],
                    in_=k_sh[r, :].rearrange("(dc p s) -> p dc s",
                                             p=128, s=512))
                nc.gpsimd.dma_start(
                    out=vp_all[:, r, :, :],
                    in_=v_sh[r, :].rearrange("(ss p f) -> p ss f",
                                             p=128, f=DP))

            # ---- phase 2: attention for our 1024 rows over all keys ----
            ps3 = tc.tile_pool(name="ps_at", bufs=1, space="PSUM")
            ps_at = ps3.__enter__()
            ps3b = tc.tile_pool(name="ps_sc", bufs=3, space="PSUM")
            ps_sc = ps3b.__enter__()

            NST = S // 128                            # 32 key tiles

            def scores(p, i):
                blk, st = i // 4, i % 4
                sc = ps_sc.tile([128, 512], F32, tag="sc")
                for dc in range(2):
                    nc.tensor.matmul(
                        sc, kT_all[:, blk, dc, st * 128:(st + 1) * 128],
                        qT_sb[:, dc, p * 512:(p + 1) * 512],
                        start=(dc == 0), stop=(dc == 1))
                ex = epool.tile([128, 512], BF16, tag="ex")
                nc.scalar.activation(out=ex, in_=sc, func=AF.Exp,
                                     bias=maskb_sb[:, i:i + 1], scale=SCALE)
                return ex

            y_r = y.rearrange("(c t p) d -> c p t d", c=2, p=128)
            for p in range(2):                        # q chunks of 512 rows
                at = [ps_at.tile([128, DP], F32, tag=f"at{i}", name=f"at{i}")
                      for i in range(4)]
                ex = [None] * NST

                def av(i):
                    blk, st = i // 4, i % 4
                    for qt in range(4):
                        nc.tensor.matmul(
                            at[qt], ex[i][:, qt * 128:(qt + 1) * 128],
                            vp_all[:, blk, st, :],
                            start=(i == 0), stop=(i == NST - 1))

                ex[0] = scores(p, 0)
                for i in range(1, NST):
                    ex[i] = scores(p, i)
                    av(i - 1)
                av(NST - 1)

                # ---- epilogue straight out of PSUM: normalize, residual,
                # LayerNorm ----
                h_half = hpool.tile([128, 4, D], F32, tag="h")
                for qt in range(4):
                    hs = h_half[:, qt, :]
                    rec = small.tile([128, 1], F32, tag="rec")
                    nc.vector.reciprocal(out=rec, in_=at[qt][:, D:D + 1])
                    nc.vector.tensor_scalar_mul(out=hs, in0=at[qt][:, 0:D],
                                                scalar1=rec)
                    nc.vector.tensor_add(out=hs, in0=hs,
                                         in1=qres_sb[:, 4 * p + qt, :])
                    stats = small.tile([128, 6], F32, tag="stats")
                    nc.vector.bn_stats(out=stats, in_=hs)
                    mv = small.tile([128, 2], F32, tag="mv")
                    nc.vector.bn_aggr(out=mv, in_=stats)
                    rstd = small.tile([128, 1], F32, tag="rstd")
                    nc.scalar.activation(out=rstd, in_=mv[:, 1:2], func=AF.Sqrt,
                                         bias=eps_sb, scale=1.0)
                    nc.vector.reciprocal(out=rstd, in_=rstd)
                    nc.vector.tensor_scalar(out=hs, in0=hs,
                                            scalar1=mv[:, 0:1], scalar2=rstd,
                                            op0=ALU.subtract, op1=ALU.mult)
                    nc.vector.tensor_mul(out=hs, in0=hs, in1=gamma_sb)
                    nc.vector.tensor_add(out=hs, in0=hs, in1=beta_sb)
                nc.gpsimd.dma_start(out=y_r[p], in_=h_half)

            ps3b.__exit__(None, None, None)
            ps3.__exit__(None, None, None)

    nc.finalize()
    return nc


_NC_CACHE = None


def _make_in_maps(inputs):
    jq = np.asarray(inputs["justice_queries"], dtype=np.float32)
    x = np.asarray(inputs["chunk_embeddings"], dtype=np.float32)[0]
    mask = np.asarray(inputs["chunk_mask"])
    wkv = np.asarray(inputs["W_kv"], dtype=np.float32)
    wout = np.asarray(inputs["W_out"], dtype=np.float32)
    gamma = np.asarray(inputs["ln_gamma"], dtype=np.float32)
    beta = np.asarray(inputs["ln_beta"], dtype=np.float32)

    import ml_dtypes
    bf16 = ml_dtypes.bfloat16
    xT = np.ascontiguousarray(x.T.astype(bf16))         # (L, S)
    wkvT = np.ascontiguousarray(wkv.T.astype(bf16))     # (L, 2D)
    flat = np.ascontiguousarray(jq.reshape(J * Q, D))   # (8192, D)
    qT = np.ascontiguousarray(flat.T.astype(bf16))      # (D, 8192)
    woutT = np.ascontiguousarray(wout.T.astype(bf16))   # (D, D)
    mb_full = np.where(mask != 0, 0.0, -1e30).astype(np.float32)
    mb = np.ascontiguousarray(mb_full.reshape(S // 128, 128).T)

    in_maps = []
    for c in range(N_CORES):
        in_maps.append({
            "xT": np.ascontiguousarray(xT[:, c * S_LOC:(c + 1) * S_LOC]),
            "wkvT": wkvT,
            "qT": np.ascontiguousarray(qT[:, c * QR:(c + 1) * QR]),
            "qres": np.ascontiguousarray(flat[c * QR:(c + 1) * QR, :]),
            "woutT": woutT,
            "maskb": mb,
            "gamma": gamma,
            "beta": beta,
        })
    return in_maps


def kernel(**inputs) -> np.ndarray:
    global _NC_CACHE
    in_maps = _make_in_maps(inputs)
    if _NC_CACHE is None:
        _NC_CACHE = build_program()
    res = run_bass_kernel_spmd(_NC_CACHE, in_maps, list(range(N_CORES)))
    out = np.concatenate([res.results[c]["y"] for c in range(N_CORES)], axis=0)
    return np.ascontiguousarray(out.reshape(J, Q, D).astype(np.float32))


# revision 22
# speedup vs baseline: 1.3732x; 1.1257x over previous
"""ChunkCrossAttention Trainium2 kernel.

Math (per reference):
  x = chunk_embeddings[0]                      # (S, L)
  k, v = split(x @ W_kv.T)                     # (S, D) each
  scores = einsum('jqd,sd->jqs', q, k) / sqrt(D), masked
  attn = softmax(scores, -1)
  out = (attn @ v) @ W_out.T + q  -> LayerNorm(gamma, beta)

Strategy (8 NeuronCores) — AllGather-KV:
  - KV projection sharded over S: each core projects its own 512 keys
    (k^T, v^T in [d, s] layout straight out of the PE).
  - W_out folded into v: v' = v @ W_out.T, with two ones columns
    appended so the attention matmul also emits the softmax denominator.
  - Each core writes its K^T / v' block (526 KB bf16) to DRAM and ONE
    AllGather (Shared pair-HBM output = the fast collective path)
    replicates all 4096 keys everywhere.  This is the only collective:
    partial-sum exchange, cross-core reduction and their per-collective
    latency floors are gone entirely.
  - Each core then attends its own 1024 query rows over all 4096 keys
    (same FLOPs as the key-sharded variant), so softmax normalization,
    residual and LayerNorm are purely local.  Gathered K/V blocks are
    DMA'd to SBUF per key-block, so attention starts as soon as the
    first block lands.
  - Softmax runs without max-subtraction (scores ~ N(0,1), exp is safe
    in f32), mask folded into the Exp bias.
  - Attention inner loop is software-pipelined (sc_i then av_{i-1}) so
    the PE never waits on the scalar engine's Exp.
"""
import sys

sys.path.insert(0, "/opt/trn_rl_repo")

import numpy as np

import concourse.bacc as bacc
import concourse.mybir as mybir
import concourse.tile as tile
from concourse.bass_utils import run_bass_kernel_spmd

N_CORES = 8
J, Q, D = 64, 128, 256
S, L = 4096, 4096
S_LOC = S // N_CORES          # 512 keys per core
QALL = J * Q                  # 8192 query rows total
QR = QALL // N_CORES          # 1024 query rows per core (output shard)
DP = D + 2                    # attention free dim: D outputs + denom + pad
KELEM = 2 * 128 * 512         # K^T elems in the kv blob
VELEM = 4 * 128 * DP          # v' elems in the kv blob
LN_EPS = 1e-5
SCALE = 1.0 / np.sqrt(D)

F32 = mybir.dt.float32
BF16 = mybir.dt.bfloat16
AF = mybir.ActivationFunctionType
ALU = mybir.AluOpType


def build_program():
    nc = bacc.Bacc(None, num_devices=N_CORES)

    xT = nc.declare_dram_parameter("xT", [L, S_LOC], BF16, isOutput=False)
    wkvT = nc.declare_dram_parameter("wkvT", [L, 2 * D], BF16, isOutput=False)
    qT = nc.declare_dram_parameter("qT", [D, QR], BF16, isOutput=False)
    qres = nc.declare_dram_parameter("qres", [QR, D], F32, isOutput=False)
    woutT = nc.declare_dram_parameter("woutT", [D, D], BF16, isOutput=False)
    maskb = nc.declare_dram_parameter("maskb", [128, S // 128], F32,
                                      isOutput=False)
    gamma = nc.declare_dram_parameter("gamma", [D], F32, isOutput=False)
    beta = nc.declare_dram_parameter("beta", [D], F32, isOutput=False)
    y = nc.declare_dram_parameter("y", [QR, D], F32, isOutput=True)

    k_loc = nc.dram_tensor("k_loc", [KELEM], BF16)
    k_sh = nc.dram_tensor("k_sh", [N_CORES, KELEM], BF16,
                          addr_space="Shared")
    v_loc = nc.dram_tensor("v_loc", [VELEM], BF16)
    v_sh = nc.dram_tensor("v_sh", [N_CORES, VELEM], BF16,
                          addr_space="Shared")
    # tiny warmup AllGather: its doorbell rings at ~13us so the collective
    # stream's one-time setup cost burns during phase 1, not after it
    warm_loc = nc.dram_tensor("warm_loc", [128], BF16)
    warm_sh = nc.dram_tensor("warm_sh", [N_CORES, 128], BF16,
                             addr_space="Shared")

    import concourse.bass as bass

    with tile.TileContext(nc) as tc:
        with tc.tile_pool(name="singles", bufs=1) as singles, \
             tc.tile_pool(name="xw", bufs=6) as xw, \
             tc.tile_pool(name="kv", bufs=1) as kvp, \
             tc.tile_pool(name="exp", bufs=4) as epool, \
             tc.tile_pool(name="exp0", bufs=33) as epool0, \
             tc.tile_pool(name="hpool", bufs=2) as hpool, \
             tc.tile_pool(name="small", bufs=8) as small:

            # ---- constants + per-core q inputs (gpsimd queue) ----
            woutT_sb = singles.tile([128, 2, D], BF16)
            nc.gpsimd.dma_start(out=woutT_sb,
                                in_=woutT.rearrange("(dc p) d2 -> p dc d2",
                                                    p=128))
            maskb_sb = singles.tile([128, S // 128], F32)
            nc.gpsimd.dma_start(out=maskb_sb, in_=maskb[:, :])
            g_ap = gamma[:]
            gamma_sb = singles.tile([128, D], F32)
            nc.gpsimd.dma_start(out=gamma_sb, in_=bass.AP(
                tensor=g_ap.tensor, offset=g_ap.offset,
                ap=[[0, 128], g_ap.ap[0]]))
            b_ap = beta[:]
            beta_sb = singles.tile([128, D], F32)
            nc.gpsimd.dma_start(out=beta_sb, in_=bass.AP(
                tensor=b_ap.tensor, offset=b_ap.offset,
                ap=[[0, 128], b_ap.ap[0]]))
            eps_sb = singles.tile([128, 1], F32)
            nc.vector.memset(eps_sb, LN_EPS)
            warm_sb = small.tile([128, 128], BF16, tag="warm")
            nc.vector.memset(warm_sb, 0.0)
            nc.sync.dma_start(out=warm_loc[:], in_=warm_sb[0:1, :])
            nc.gpsimd.collective_compute(
                "AllGather", ALU.bypass,
                replica_groups=[list(range(N_CORES))],
                ins=[warm_loc[:]], outs=[warm_sh[:, :]])
            qT_sb = singles.tile([128, 2, QR], BF16)
            qres_sb = singles.tile([128, QR // 128, D], F32)

            # ---- phase 1: local K^T / V^T projection over the S shard ----
            # x on the sync queue, w on the scalar queue; first chunk small
            # so the PE starts as early as possible.
            ps1 = tc.tile_pool(name="ps_kv", bufs=1, space="PSUM")
            ps_kv = ps1.__enter__()
            acc = [ps_kv.tile([128, S_LOC], F32, tag=f"acc{h}", name=f"acc{h}")
                   for h in range(4)]
            chunks = [(0, 128), (128, 384)] + [(512 * i, 512)
                                               for i in range(1, 8)]
            n_mm = sum(nr // 128 for _, nr in chunks) * 4
            mm_i = 0
            for (r0, nr) in chunks:
                na = nr // 128
                xt = xw.tile([128, na, S_LOC], BF16, tag=f"xt{na}")
                nc.sync.dma_start(
                    out=xt,
                    in_=xT[r0:r0 + nr, :].rearrange("(a p) s -> p a s", p=128))
                wt = xw.tile([128, na, 2 * D], BF16, tag=f"wt{na}")
                nc.scalar.dma_start(
                    out=wt,
                    in_=wkvT[r0:r0 + nr, :].rearrange("(a p) s -> p a s",
                                                      p=128))
                for a in range(na):
                    for h in range(4):
                        nc.tensor.matmul(acc[h], wt[:, a, h * 128:(h + 1) * 128],
                                         xt[:, a, :], start=(mm_i == 0),
                                         stop=(mm_i == n_mm - 4 + h))
                    mm_i += 4
                if r0 == 512 * 4:
                    # q inputs pinned behind the mid-stream x chunk so they
                    # don't steal HBM bandwidth from phase 1 (RAW on xt pins
                    # the probe, WAR on the target pins the transfer)
                    probe = small.tile([128, 1], BF16, tag="probe")
                    nc.vector.tensor_add(out=probe, in0=qT_sb[:, 0, 0:1],
                                         in1=xt[:, 0, 0:1])
                    nc.gpsimd.dma_start(
                        out=qT_sb,
                        in_=qT.rearrange("(dc p) q -> p dc q", p=128))
                    qprobe = small.tile([128, 1], F32, tag="qprobe")
                    nc.vector.tensor_add(out=qprobe, in0=qres_sb[:, 0, 0:1],
                                         in1=xt[:, 0, 0:1])
                    nc.gpsimd.dma_start(
                        out=qres_sb,
                        in_=qres.rearrange("(t p) d -> p t d", p=128))

            kT_loc = kvp.tile([128, 2, S_LOC], BF16)
            nc.scalar.copy(out=kT_loc[:, 0, :], in_=acc[0])
            nc.scalar.copy(out=kT_loc[:, 1, :], in_=acc[1])
            # publish + AllGather K immediately — scores only need K, so
            # this collective runs while v' is still being folded
            nc.sync.dma_start(
                out=k_loc[:].rearrange("(dc p s) -> p dc s", p=128, s=512),
                in_=kT_loc)
            nc.gpsimd.collective_compute(
                "AllGather", ALU.bypass,
                replica_groups=[list(range(N_CORES))],
                ins=[k_loc[:]], outs=[k_sh[:, :]])

            vT_loc = kvp.tile([128, 2, S_LOC], BF16)
            nc.scalar.copy(out=vT_loc[:, 0, :], in_=acc[2])
            nc.scalar.copy(out=vT_loc[:, 1, :], in_=acc[3])

            # ---- v' = v @ W_out.T, plus ones columns -> [s, DP] ----
            vp_sb = kvp.tile([128, 4, DP], BF16)
            nc.vector.memset(vp_sb, 1.0)
            for ss in range(4):
                pv = ps_kv.tile([128, D], F32, tag="pv", name="pv")
                for dc in range(2):
                    nc.tensor.matmul(
                        pv, vT_loc[:, dc, ss * 128:(ss + 1) * 128],
                        woutT_sb[:, dc, :], start=(dc == 0), stop=(dc == 1))
                nc.vector.tensor_copy(out=vp_sb[:, ss, 0:D], in_=pv)
            ps1.__exit__(None, None, None)

            nc.sync.dma_start(
                out=v_loc[:].rearrange("(ss p f) -> p ss f", p=128, f=DP),
                in_=vp_sb)
            nc.gpsimd.collective_compute(
                "AllGather", ALU.bypass,
                replica_groups=[list(range(N_CORES))],
                ins=[v_loc[:]], outs=[v_sh[:, :]])

            # gathered K/v' -> SBUF, one DMA pair per key-block so the
            # attention pipeline starts on block 0 immediately
            kT_all = kvp.tile([128, N_CORES, 2, 512], BF16)
            vp_all = kvp.tile([128, N_CORES, 4, DP], BF16)
            for r in range(N_CORES):
                nc.sync.dma_start(
                    out=kT_all[:, r, :, :],
                    in_=k_sh[r, :].rearrange("(dc p s) -> p dc s",
                                             p=128, s=512))
                nc.gpsimd.dma_start(
                    out=vp_all[:, r, :, :],
                    in_=v_sh[r, :].rearrange("(ss p f) -> p ss f",
                                             p=128, f=DP))

            # ---- phase 2: attention for our 1024 rows over all keys ----
            ps3 = tc.tile_pool(name="ps_at", bufs=1, space="PSUM")
            ps_at = ps3.__enter__()
            ps3b = tc.tile_pool(name="ps_sc", bufs=3, space="PSUM")
            ps_sc = ps3b.__enter__()

            NST = S // 128                            # 32 key tiles

            def scores(p, i, pool):
                blk, st = i // 4, i % 4
                sc = ps_sc.tile([128, 512], F32, tag="sc")
                for dc in range(2):
                    nc.tensor.matmul(
                        sc, kT_all[:, blk, dc, st * 128:(st + 1) * 128],
                        qT_sb[:, dc, p * 512:(p + 1) * 512],
                        start=(dc == 0), stop=(dc == 1))
                ex = pool.tile([128, 512], BF16, tag="ex")
                nc.scalar.activation(out=ex, in_=sc, func=AF.Exp,
                                     bias=maskb_sb[:, i:i + 1], scale=SCALE)
                return ex

            y_r = y.rearrange("(c t p) d -> c p t d", c=2, p=128)
            for p in range(2):                        # q chunks of 512 rows
                at = [ps_at.tile([128, DP], F32, tag=f"at{i}", name=f"at{i}")
                      for i in range(4)]
                ex = [None] * NST

                def av(i):
                    blk, st = i // 4, i % 4
                    for qt in range(4):
                        nc.tensor.matmul(
                            at[qt], ex[i][:, qt * 128:(qt + 1) * 128],
                            vp_all[:, blk, st, :],
                            start=(i == 0), stop=(i == NST - 1))

                if p == 0:
                    # all scores first: they only need K, so they stream
                    # while the v' AllGather is still completing
                    for i in range(NST):
                        ex[i] = scores(p, i, epool0)
                    for i in range(NST):
                        av(i)
                else:
                    ex[0] = scores(p, 0, epool)
                    for i in range(1, NST):
                        ex[i] = scores(p, i, epool)
                        av(i - 1)
                    av(NST - 1)

                # ---- epilogue straight out of PSUM: normalize, residual,
                # LayerNorm ----
                h_half = hpool.tile([128, 4, D], F32, tag="h")
                for qt in range(4):
                    hs = h_half[:, qt, :]
                    rec = small.tile([128, 1], F32, tag="rec")
                    nc.vector.reciprocal(out=rec, in_=at[qt][:, D:D + 1])
                    nc.vector.tensor_scalar_mul(out=hs, in0=at[qt][:, 0:D],
                                                scalar1=rec)
                    nc.vector.tensor_add(out=hs, in0=hs,
                                         in1=qres_sb[:, 4 * p + qt, :])
                    stats = small.tile([128, 6], F32, tag="stats")
                    nc.vector.bn_stats(out=stats, in_=hs)
                    mv = small.tile([128, 2], F32, tag="mv")
                    nc.vector.bn_aggr(out=mv, in_=stats)
                    rstd = small.tile([128, 1], F32, tag="rstd")
                    nc.scalar.activation(out=rstd, in_=mv[:, 1:2], func=AF.Sqrt,
                                         bias=eps_sb, scale=1.0)
                    nc.vector.reciprocal(out=rstd, in_=rstd)
                    nc.vector.tensor_scalar(out=hs, in0=hs,
                                            scalar1=mv[:, 0:1], scalar2=rstd,
                                            op0=ALU.subtract, op1=ALU.mult)
                    nc.vector.tensor_mul(out=hs, in0=hs, in1=gamma_sb)
                    nc.vector.tensor_add(out=hs, in0=hs, in1=beta_sb)
                nc.gpsimd.dma_start(out=y_r[p], in_=h_half)

            ps3b.__exit__(None, None, None)
            ps3.__exit__(None, None, None)

    nc.finalize()
    return nc


_NC_CACHE = None


def _make_in_maps(inputs):
    jq = np.asarray(inputs["justice_queries"], dtype=np.float32)
    x = np.asarray(inputs["chunk_embeddings"], dtype=np.float32)[0]
    mask = np.asarray(inputs["chunk_mask"])
    wkv = np.asarray(inputs["W_kv"], dtype=np.float32)
    wout = np.asarray(inputs["W_out"], dtype=np.float32)
    gamma = np.asarray(inputs["ln_gamma"], dtype=np.float32)
    beta = np.asarray(inputs["ln_beta"], dtype=np.float32)

    import ml_dtypes
    bf16 = ml_dtypes.bfloat16
    xT = np.ascontiguousarray(x.T.astype(bf16))         # (L, S)
    wkvT = np.ascontiguousarray(wkv.T.astype(bf16))     # (L, 2D)
    flat = np.ascontiguousarray(jq.reshape(J * Q, D))   # (8192, D)
    qT = np.ascontiguousarray(flat.T.astype(bf16))      # (D, 8192)
    woutT = np.ascontiguousarray(wout.T.astype(bf16))   # (D, D)
    mb_full = np.where(mask != 0, 0.0, -1e30).astype(np.float32)
    mb = np.ascontiguousarray(mb_full.reshape(S // 128, 128).T)

    in_maps = []
    for c in range(N_CORES):
        in_maps.append({
            "xT": np.ascontiguousarray(xT[:, c * S_LOC:(c + 1) * S_LOC]),
            "wkvT": wkvT,
            "qT": np.ascontiguousarray(qT[:, c * QR:(c + 1) * QR]),
            "qres": np.ascontiguousarray(flat[c * QR:(c + 1) * QR, :]),
            "woutT": woutT,
            "maskb": mb,
            "gamma": gamma,
            "beta": beta,
        })
    return in_maps


def kernel(**inputs) -> np.ndarray:
    global _NC_CACHE
    in_maps = _make_in_maps(inputs)
    if _NC_CACHE is None:
        _NC_CACHE = build_program()
    res = run_bass_kernel_spmd(_NC_CACHE, in_maps, list(range(N_CORES)))
    out = np.concatenate([res.results[c]["y"] for c in range(N_CORES)], axis=0)
    return np.ascontiguousarray(out.reshape(J, Q, D).astype(np.float32))
